# revision 1
# baseline (speedup 1.0000x reference)
"""Trainium2 Bass kernel for causal self-attention with QK RMS-norm + rotary.

Full (unsharded) inputs in, full output out.  Internally sharded over 8
NeuronCores: data parallel on batch (2) x tensor parallel on head groups
(16 heads -> 4 groups of 4).  Each core computes q/k/v for its 4 heads on
its batch, causal flash-style attention, and a partial output projection
(its 512-column slice of Wp's input dim); the host sums the 4 partials per
batch ("all-reduce after proj" done host-side) and adds the output bias.

Per-core pipeline (single Bass program, SPMD over 8 cores):
  Phase 1, per 128-row t-tile: QKV projections with x^T tiles as the
    stationary matmul operand (q/k/v share each weight load); rotary applied
    to raw q/k straight out of PSUM (rotation commutes with RMS-norm's
    per-row scale); per-head squared-norm via fused multiply+reduce; q
    normalized in place; k's norm folded into the softmax exp scale
    (exp(scale*s) with per-partition scale = 0.12/rms(k_j)); q^T/k^T built
    with PE transposes and spilled to DRAM scratch so SBUF holds only v +
    weights.
  Phase 2, per 512-column query chunk, per head: scores^T = k_tile^T-block
    @ q^T computed [j,i]-transposed so the softmax denominator comes from a
    ones-stationary matmul (every PSUM partition = sum_j p) and attn@v needs
    no transpose of p; exp on ScalarE; causal mask on diagonal blocks via
    GpSimd affine_select (upper-triangle blocks never computed); y^T
    accumulated in PSUM over j-tiles, normalized by reciprocal(l); output
    projection contracts the 4 head-slices of y^T against Wp^T.

All matmuls run as float32r (fp32 data, FP22-truncated multiply at full
1 cycle/row PE rate for moving dims >= 256) with fp32 PSUM accumulation.
"""

import os
import sys

import numpy as np

try:
    import concourse.bass as bass
except ImportError:  # fall back to the repo checkout baked into the image
    for _p in ("/opt/trn_rl_repo", "/root/.axon_site/_ro/trn_rl_repo"):
        if os.path.isdir(_p) and _p not in sys.path:
            sys.path.append(_p)
    import concourse.bass as bass

import concourse.mybir as mybir
import concourse.tile as tile
from concourse.bass_utils import run_bass_kernel_spmd
from concourse.masks import make_identity
from concourse.vector_clock import ScopedClock

F32 = mybir.dt.float32
F32R = mybir.dt.float32r
AF = mybir.ActivationFunctionType

DIM = 2048
HEAD_DIM = 128
NUM_HEADS = 16
B, T = 2, 2048
EPS = 1.1920929e-07
SCALE = 0.12

NCORES = 8
HG = 4                    # heads per core
GD = HG * HEAD_DIM        # 512: per-core q/k/v width and Wp input slice
NT = T // 128             # 16 t-tiles
ND = DIM // 128           # 16 contraction tiles
NI = T // 512             # 4 query chunks
P = 128


class _TC(tile.TileContext):
    """TileContext whose final drain splits its semaphore waits across
    single-wait NOPs -- the walrus build in this image rejects CTRL
    instructions carrying 3+ sync waits ("Too many sync wait commands")."""

    def _drain_and_barrier(self, tick_clock, wait_clock):
        probe = self.nc.sync.nop(nofuse=True)
        wait_clock.add_sem_waits(probe.ins, ScopedClock({None: tick_clock.global_clock}))
        si = probe.ins.sync_info
        waits = list(si.on_wait) if si and si.on_wait else []
        if si is not None and si.on_wait:
            del si.on_wait[1:]
        for w in waits[1:]:
            nop = self.nc.sync.nop(nofuse=True)
            nsi = nop.ins.sync_info
            if nsi is None:
                nop.ins.sync_info = mybir.SyncInfo(on_wait=[w], on_update=[])
            else:
                nsi.on_wait.append(w)
        self.nc.sync.drain()
        self.nc.all_engine_barrier()
        assert self.sems is not None
        popped = self.nc._tile_sem_poison_stack.pop()
        assert popped is self._sem_poison
        self.nc.clear_and_free_semaphores(list(self.sems.allocated().values()))
        self.nc.all_engine_barrier()


def _r(ap):
    return ap.bitcast(F32R)


_MAX_WAITS = 1


def _split_excess_waits(nc, maxw=_MAX_WAITS):
    """The walrus build in this image rejects instructions with >1 sync
    waits; spill extra waits onto NoOps inserted just before the offender
    on the same engine (all waits are preconditions, so order is free)."""
    n = 0
    for f in nc.m.functions:
        for bb in f.blocks:
            out = []
            for inst in bb.instructions:
                si = inst.sync_info
                waits = list(si.on_wait) if si and si.on_wait else []
                if len(waits) > maxw:
                    extra = waits[:-maxw]
                    del si.on_wait[: len(extra)]
                    for i in range(0, len(extra), maxw):
                        n += 1
                        nop = mybir.InstNoOp(name=f"I-wsplit-{n}-{inst.name}",
                                             ins=[], outs=[])
                        nop.engine = inst.engine
                        nop.sync_info = mybir.SyncInfo(
                            on_wait=extra[i:i + maxw], on_update=[])
                        out.append(nop)
                out.append(inst)
            bb.instructions[:] = out


def _build_nc(has_qkv_bias: bool):
    nc = bass.Bass("TRN2", target_bir_lowering=False, debug=False, num_devices=NCORES)

    xt = nc.dram_tensor("xt", [DIM, T], F32R, kind="ExternalInput")
    wqt = nc.dram_tensor("wqt", [DIM, GD], F32R, kind="ExternalInput")
    wkt = nc.dram_tensor("wkt", [DIM, GD], F32R, kind="ExternalInput")
    wvt = nc.dram_tensor("wvt", [DIM, GD], F32R, kind="ExternalInput")
    wpt = nc.dram_tensor("wpt", [GD, DIM], F32R, kind="ExternalInput")
    cosb = nc.dram_tensor("cosb", [T, GD], F32, kind="ExternalInput")
    onesd = nc.dram_tensor("onesd", [P, P], F32R, kind="ExternalInput")
    sinb = nc.dram_tensor("sinb", [T, GD], F32, kind="ExternalInput")
    if has_qkv_bias:
        bq = nc.dram_tensor("bq", [GD], F32, kind="ExternalInput")
        bk = nc.dram_tensor("bk", [GD], F32, kind="ExternalInput")
        bv = nc.dram_tensor("bv", [GD], F32, kind="ExternalInput")
    out = nc.dram_tensor("out", [T, DIM], F32, kind="ExternalOutput")

    xt_v = xt.rearrange("(do p) t -> p do t", p=P)      # [128, 16, 2048]
    wqt_v = wqt.rearrange("(do p) o -> p do o", p=P)    # [128, 16, 512]
    wkt_v = wkt.rearrange("(do p) o -> p do o", p=P)
    wvt_v = wvt.rearrange("(do p) o -> p do o", p=P)
    wpt_v = wpt.rearrange("(co p) o -> p co o", p=P)    # [128, 4, 2048]

    with _TC(nc) as tc:
        with (
            tc.tile_pool(name="const", bufs=1) as constp,
            tc.tile_pool(name="persist", bufs=1) as persist,
            tc.tile_pool(name="dram", bufs=1, space="DRAM") as dramp,
        ):
            identity = constp.tile([P, P], F32)
            make_identity(nc, identity)
            ones_t = constp.tile([P, P], F32R)
            nc.sync.dma_start(out=ones_t, in_=onesd[:, :])
            eps_t = constp.tile([P, 1], F32)
            nc.vector.memset(eps_t, EPS)
            if has_qkv_bias:
                bias_b = constp.tile([P, 3, GD], F32)
                for bi, bten in enumerate((bq, bk, bv)):
                    bcast = bass.AP(tensor=bten.tensor, offset=bten.offset,
                                    ap=[[0, P]] + list(bten.ap))
                    nc.sync.dma_start(out=bias_b[:, bi, :], in_=bcast)

            v_sb = persist.tile([P, NT, GD], F32R)       # v, natural [t, head*128]
            rk_sb = persist.tile([P, NT, HG], F32)      # 0.12/rms(k) per (t, head)
            qt_dram = dramp.tile([P, NT, HG, P], F32R)   # q^T spill [c, tt, h, t]
            kt_dram = dramp.tile([P, NT, HG, P], F32R)

            # ---------------- Phase 1: QKV + rotary + norms + spills ----------
            # K/V for t-tile i and Q for t-tile i-1 per iteration: Q trails
            # one tile so the PE starts on K/V as soon as the first weight
            # chunks land instead of waiting for all three weight matrices.
            with (
                tc.tile_pool(name="wqkv", bufs=1) as wpool,
                tc.tile_pool(name="ph1", bufs=3) as ph1,
                tc.tile_pool(name="ph1t", bufs=2) as ph1t,
                tc.tile_pool(name="pp1", bufs=1, space="PSUM") as pp1,
            ):
                wq_sb = wpool.tile([P, ND, GD], F32R)
                wk_sb = wpool.tile([P, ND, GD], F32R)
                wv_sb = wpool.tile([P, ND, GD], F32R)

                acts = {}

                def load_acts(tt):
                    xtile = ph1.tile([P, ND, P], F32R, tag="xtile",
                                     name=f"xtile{tt}")
                    nc.sync.dma_start(out=xtile,
                                      in_=xt_v[:, :, tt * P:(tt + 1) * P])
                    ctile = ph1.tile([P, GD], F32, tag="ctile", name=f"ctile{tt}")
                    stile = ph1.tile([P, GD], F32, tag="stile", name=f"stile{tt}")
                    nc.sync.dma_start(out=ctile, in_=cosb[tt * P:(tt + 1) * P, :])
                    nc.sync.dma_start(out=stile, in_=sinb[tt * P:(tt + 1) * P, :])
                    acts[tt] = (xtile, ctile, stile)

                load_acts(0)
                # d-interleaved so tile 0's K/V contraction can begin after
                # the first chunks arrive; Q chunks trail by design.
                for d in range(ND):
                    nc.sync.dma_start(out=wk_sb[:, d, :], in_=wkt_v[:, d, :])
                    nc.sync.dma_start(out=wv_sb[:, d, :], in_=wvt_v[:, d, :])
                    nc.sync.dma_start(out=wq_sb[:, d, :], in_=wqt_v[:, d, :])

                def qk_dve(which, src, tt):
                    """Rotary + rms stats on DVE/ACT; returns the rotated tile."""
                    _, ctile, stile = acts[tt]
                    u = ph1t.tile([P, HG, 2, 64], F32, tag="u")
                    w = ph1t.tile([P, HG, 2, 64], F32, tag="w")
                    nc.vector.tensor_mul(u.rearrange("p h x y -> p (h x y)"), src, ctile)
                    nc.vector.tensor_mul(w.rearrange("p h x y -> p (h x y)"), src, stile)
                    rot = ph1t.tile([P, HG, P], F32, tag=which + "rot")
                    r3 = rot.rearrange("p h (x y) -> p h x y", x=2)
                    nc.vector.tensor_add(r3[:, :, 0, :], u[:, :, 0, :], w[:, :, 1, :])
                    nc.vector.tensor_sub(r3[:, :, 1, :], u[:, :, 1, :], w[:, :, 0, :])

                    ms = ph1t.tile([P, HG], F32, tag=which + "ms")
                    sq = ph1t.tile([P, HG, P], F32, tag="sq")
                    rflat = rot.rearrange("p h c -> p (h c)")
                    nc.vector.tensor_mul(
                        sq.rearrange("p h c -> p (h c)"), rflat, rflat)
                    nc.vector.reduce_sum(out=ms, in_=sq,
                                         axis=mybir.AxisListType.X)
                    rstd = ph1t.tile([P, HG], F32, tag=which + "rstd")
                    nc.scalar.activation(out=rstd, in_=ms, func=AF.Sqrt,
                                         scale=1.0 / HEAD_DIM, bias=eps_t[:, 0:1])
                    if which == "q":
                        nc.vector.reciprocal(out=rstd, in_=rstd)
                        for h in range(HG):
                            nc.vector.tensor_scalar_mul(
                                rot[:, h, :], rot[:, h, :], rstd[:, h:h + 1])
                    else:
                        nc.vector.reciprocal(out=rk_sb[:, tt, :], in_=rstd)
                        nc.vector.tensor_scalar_mul(
                            rk_sb[:, tt, :], rk_sb[:, tt, :], SCALE)
                    return rot

                def qk_transpose(which, rot, tt):
                    stage = ph1t.tile([P, HG, P], F32, tag=which + "stage")
                    for h in range(HG):
                        ptr = pp1.tile([P, P], F32, tag="ptr", bufs=2)
                        nc.tensor.transpose(ptr, rot[:, h, :], identity)
                        nc.vector.tensor_copy(out=stage[:, h, :], in_=ptr)
                    dst = qt_dram if which == "q" else kt_dram
                    nc.sync.dma_start(out=dst[:, tt, :, :].bitcast(F32), in_=stage)

                # Software pipeline: per iteration i the PE runs K/V(i) then
                # Q(i-1), then transposes k(i) (whose DVE chain overlapped
                # the Q matmuls) and q(i-2) (chain finished an iteration ago).
                qrots = {}
                for i in range(NT + 2):
                    if 2 <= i:
                        qk_transpose("q", qrots.pop(i - 2), i - 2)
                    if i < NT:
                        if i + 1 < NT:
                            load_acts(i + 1)
                        xtile = acts[i][0]
                        ps_k = pp1.tile([P, GD], F32, tag="psk", bufs=2)
                        ps_v = pp1.tile([P, GD], F32, tag="psv", bufs=2)
                        for d in range(ND):
                            st, sp = d == 0, d == ND - 1
                            nc.tensor.matmul(ps_k, xtile[:, d, :], wk_sb[:, d, :],
                                             start=st, stop=sp)
                            nc.tensor.matmul(ps_v, xtile[:, d, :], wv_sb[:, d, :],
                                             start=st, stop=sp)
                        if has_qkv_bias:
                            kb = ph1t.tile([P, GD], F32, tag="kb")
                            nc.vector.tensor_add(kb, ps_k, bias_b[:, 1, :])
                            nc.vector.tensor_add(v_sb[:, i, :], ps_v, bias_b[:, 2, :])
                            src_k = kb
                        else:
                            nc.vector.tensor_copy(out=v_sb[:, i, :], in_=ps_v)
                            src_k = ps_k
                        krot = qk_dve("k", src_k, i)
                    if 1 <= i <= NT:
                        tt = i - 1
                        xtile = acts[tt][0]
                        ps_q = pp1.tile([P, GD], F32, tag="psq", bufs=2)
                        for d in range(ND):
                            nc.tensor.matmul(ps_q, xtile[:, d, :], wq_sb[:, d, :],
                                             start=d == 0, stop=d == ND - 1)
                        if has_qkv_bias:
                            qb = ph1t.tile([P, GD], F32, tag="qb")
                            nc.vector.tensor_add(qb, ps_q, bias_b[:, 0, :])
                            src_q = qb
                        else:
                            src_q = ps_q
                        qrots[tt] = qk_dve("q", src_q, tt)
                    if i < NT:
                        qk_transpose("k", krot, i)

            # ---------------- Phase 2: attention + projection -----------------
            with (
                tc.tile_pool(name="ph2w", bufs=1) as ph2w,
                tc.tile_pool(name="ph2", bufs=2) as ph2,
                tc.tile_pool(name="pp2", bufs=1, space="PSUM") as pp2,
            ):
                # Phase-boundary DMAs go on the idle gpsimd SWDGE queue — the
                # sync queue is still draining late phase-1 spills.  Head 0's
                # k^T first (it gates the first scores matmul); Wp last on
                # sync (not needed until the first projection, much later).
                kt_sb = ph2w.tile([P, HG, T], F32R)
                nc.gpsimd.dma_start(
                    out=kt_sb[:, 0, :].rearrange("p (n t) -> p n t", t=P),
                    in_=kt_dram[:, :, 0, :])
                wp_sb = ph2w.tile([P, HG, DIM], F32R)

                def proj_half(ic, y_sb, half):
                    for it in (half, 2 + half):
                        for dc in range(4):
                            ps_o = pp2.tile([P, 512], F32, tag="po", bufs=2)
                            for co in range(HG):
                                nc.tensor.matmul(
                                    ps_o, y_sb[:, co, it * P:(it + 1) * P],
                                    wp_sb[:, co, dc * 512:(dc + 1) * 512],
                                    start=co == 0, stop=co == HG - 1)
                            o_sb = ph2.tile([P, 512], F32, tag="o", bufs=3)
                            nc.vector.tensor_copy(out=o_sb, in_=ps_o)
                            nc.sync.dma_start(
                                out=out[(4 * ic + it) * P:(4 * ic + it + 1) * P,
                                        dc * 512:(dc + 1) * 512],
                                in_=o_sb)

                prev = None
                for ic in range(NI):
                    qT = ph2.tile([P, HG, 512], F32R, tag="qT")
                    for h in range(HG):
                        nc.gpsimd.dma_start(
                            out=qT[:, h, :].rearrange("p (n t) -> p n t", t=P),
                            in_=qt_dram[:, 4 * ic:4 * ic + 4, h, :])
                    if ic == 0:
                        for h in range(1, HG):
                            nc.gpsimd.dma_start(
                                out=kt_sb[:, h, :].rearrange("p (n t) -> p n t", t=P),
                                in_=kt_dram[:, :, h, :])
                        nc.sync.dma_start(out=wp_sb, in_=wpt_v)
                    y_sb = ph2.tile([P, HG, 512], F32R, tag="y")
                    nj = 4 * (ic + 1)
                    # Heads in interleaved pairs with scores+exp prefetched one
                    # j-step ahead: by the time the PE reaches l/pV of step j,
                    # the exp (and diagonal-block mask) of step j has had four
                    # matmuls' worth of time to finish.  Half of the previous
                    # chunk's projection is emitted at each pair boundary so
                    # the PE chews on it exactly where it would otherwise wait
                    # for the pair's PSUM accumulators to drain.
                    for hp in range(HG // 2):
                        if prev is not None:
                            proj_half(prev[0], prev[1], hp)
                        hs = (2 * hp, 2 * hp + 1)
                        ps_ys = {h: pp2.tile([P, 512], F32, tag="py", bufs=2,
                                             name=f"ps_y{h}") for h in hs}
                        ps_ls = {h: pp2.tile([P, 512], F32, tag="pl", bufs=2,
                                             name=f"ps_l{h}") for h in hs}

                        def sc_exp(jt, h):
                            ps_s = pp2.tile([P, 512], F32, tag="ps", bufs=2)
                            nc.tensor.matmul(
                                ps_s, kt_sb[:, h, jt * P:(jt + 1) * P],
                                qT[:, h, :], start=True, stop=True)
                            p_sb = ph2.tile([P, 512], F32R, tag="p", bufs=6)
                            nc.scalar.activation(
                                out=p_sb, in_=ps_s, func=AF.Exp,
                                scale=rk_sb[:, jt, h:h + 1])
                            if jt >= 4 * ic:
                                nc.gpsimd.affine_select(
                                    out=p_sb, in_=p_sb,
                                    pattern=[[1, 512]], channel_multiplier=-1,
                                    base=-P * (jt - 4 * ic),
                                    compare_op=mybir.AluOpType.is_ge, fill=0.0)
                            return p_sb

                        ps = {h: sc_exp(0, h) for h in hs}
                        for jt in range(nj):
                            nxt = {}
                            if jt + 1 < nj:
                                nxt = {h: sc_exp(jt + 1, h) for h in hs}
                            for h in hs:
                                st, sp = jt == 0, jt == nj - 1
                                nc.tensor.matmul(ps_ls[h], ones_t, ps[h],
                                                 start=st, stop=sp)
                                nc.tensor.matmul(
                                    ps_ys[h], v_sb[:, jt, h * P:(h + 1) * P],
                                    ps[h], start=st, stop=sp)
                            ps = nxt
                        for h in hs:
                            # quick copies free both PSUM banks before the
                            # (slow) reciprocal; the next pair's matmuls can
                            # proceed immediately
                            l_sb = ph2.tile([P, 512], F32, tag="l_sb")
                            nc.vector.tensor_copy(out=l_sb, in_=ps_ls[h])
                            y_raw = ph2.tile([P, 512], F32, tag="y_raw")
                            nc.vector.tensor_copy(out=y_raw, in_=ps_ys[h])
                            linv = ph2.tile([P, 512], F32, tag="linv")
                            nc.vector.reciprocal(out=linv, in_=l_sb)
                            nc.vector.tensor_mul(y_sb[:, h, :], y_raw, linv)
                    prev = (ic, y_sb)
                proj_half(prev[0], prev[1], 0)
                proj_half(prev[0], prev[1], 1)
    _split_excess_waits(nc)
    return nc


_NC_CACHE = {}
_RUN_KWARGS = {}      # test harness hook: e.g. {"trace": True}
_LAST_RESULT = None   # BassKernelResults of the most recent run


def _rotary_tables():
    freq = (1.0 / 1024.0) ** np.linspace(0.0, 1.0, HEAD_DIM // 4, dtype=np.float32)
    freq = np.concatenate([freq, np.zeros(HEAD_DIM // 4, np.float32)])
    theta = np.arange(T, dtype=np.float32)[:, None] * freq[None, :]     # [T, 64]
    cos = np.cos(theta).astype(np.float32)
    sin = np.sin(theta).astype(np.float32)
    cosb = np.tile(np.concatenate([cos, cos], axis=1), (1, HG))          # [T, 512]
    sinb = np.tile(np.concatenate([sin, sin], axis=1), (1, HG))
    return np.ascontiguousarray(cosb), np.ascontiguousarray(sinb)


def kernel(x, Wq, bq, Wk, bk, Wv, bv, Wp, bp):
    x = np.asarray(x, np.float32)
    Wq, Wk, Wv, Wp = (np.asarray(a, np.float32) for a in (Wq, Wk, Wv, Wp))
    bq, bk, bv, bp = (np.asarray(a, np.float32) for a in (bq, bk, bv, bp))

    has_bias = bool(np.any(bq) or np.any(bk) or np.any(bv))
    if has_bias not in _NC_CACHE:
        _NC_CACHE[has_bias] = _build_nc(has_bias)
    nc = _NC_CACHE[has_bias]

    cosb, sinb = _rotary_tables()
    in_maps = []
    for c in range(NCORES):
        b, g = divmod(c, NCORES // B)
        sl = slice(g * GD, (g + 1) * GD)
        m = {
            "xt": np.ascontiguousarray(x[b].T),
            "wqt": np.ascontiguousarray(Wq[sl, :].T),
            "wkt": np.ascontiguousarray(Wk[sl, :].T),
            "wvt": np.ascontiguousarray(Wv[sl, :].T),
            "wpt": np.ascontiguousarray(Wp[:, sl].T),
            "cosb": cosb,
            "sinb": sinb,
            "onesd": np.ones((P, P), np.float32),
        }
        if has_bias:
            m["bq"] = np.ascontiguousarray(bq[sl])
            m["bk"] = np.ascontiguousarray(bk[sl])
            m["bv"] = np.ascontiguousarray(bv[sl])
        in_maps.append(m)

    res = run_bass_kernel_spmd(nc, in_maps, list(range(NCORES)), **_RUN_KWARGS)
    global _LAST_RESULT
    _LAST_RESULT = res
    out = np.zeros((B, T, DIM), np.float32)
    for c in range(NCORES):
        out[c // (NCORES // B)] += res.results[c]["out"]
    out += bp[None, None, :]
    return out



# revision 3
# speedup vs baseline: 1.1358x; 1.1358x over previous
"""Trainium2 Bass kernel for causal self-attention with QK RMS-norm + rotary.

Full (unsharded) inputs in, full output out.  Internally sharded over 8
NeuronCores: data parallel on batch (2) x tensor parallel on head groups
(16 heads -> 4 groups of 4).  Each core computes q/k/v for its 4 heads on
its batch, causal flash-style attention, and a partial output projection
(its 512-column slice of Wp's input dim); the host sums the 4 partials per
batch ("all-reduce after proj" done host-side) and adds the output bias.

All matmul operands are bf16 (fp32 PSUM accumulation): same PE rate as
fp32r but with fast weight loads, half the DMA/SBUF traffic, and 2x DVE
throughput on SBUF-resident elementwise work.  Numerics validated against
the fp32 reference at ~3e-3 max-rel-err (gate 2e-2).

Per-core pipeline (single Bass program, SPMD over 8 cores):
  Phase 1, per 128-row t-tile: QKV projections with x^T tiles stationary
    (q/k/v share each weight load); ScalarE evacuates PSUM to bf16 SBUF;
    RMS stats are taken on the *raw* q/k (rotation preserves row norms) so
    they overlap the rotary; rotary touches only the 64 nonzero-frequency
    channels per head (the other half is pass-through, already in place
    from the copy); q scaled by 1/rms in place; k's norm is folded into
    the softmax exp scale (0.12/rms(k_j) per scores^T partition); q^T/k^T
    built with bf16 PE transposes and kept in SBUF (no DRAM spill).
  Phase 2, per 512-column query chunk, per head: scores^T = k^T-tile @ q^T
    so the softmax denominator comes from a ones-stationary matmul and
    attn@v needs no transpose of p; exp on ScalarE (bf16 out); causal
    diagonal blocks run at reduced moving size (only j<=i columns) with a
    GpSimd triangular affine_select on the single partial [128,128] block;
    y^T and l accumulate in PSUM over j-tiles; 1/l via the fast DVE
    reciprocal; output projection contracts the 4 head-slices of y^T
    against Wp^T, interleaved with the next chunk's attention; partial
    outputs written bf16 and reduced host-side.
"""

import os
import sys

import numpy as np

try:
    import concourse.bass as bass
except ImportError:  # fall back to the repo checkout baked into the image
    for _p in ("/opt/trn_rl_repo", "/root/.axon_site/_ro/trn_rl_repo"):
        if os.path.isdir(_p) and _p not in sys.path:
            sys.path.append(_p)
    import concourse.bass as bass

import ml_dtypes

import concourse.mybir as mybir
import concourse.tile as tile
from concourse.bass_utils import run_bass_kernel_spmd
from concourse.masks import make_identity
from concourse.vector_clock import ScopedClock

F32 = mybir.dt.float32
BF16 = mybir.dt.bfloat16
AF = mybir.ActivationFunctionType

DIM = 2048
HEAD_DIM = 128
NUM_HEADS = 16
B, T = 2, 2048
EPS = 1.1920929e-07
SCALE = 0.12

NCORES = 8
HG = 4                    # heads per core
GD = HG * HEAD_DIM        # 512: per-core q/k/v width and Wp input slice
NT = T // 128             # 16 t-tiles
ND = DIM // 128           # 16 contraction tiles
NI = T // 512             # 4 query chunks
P = 128
NFREQ = HEAD_DIM // 4     # 32 nonzero-frequency channels per head


class _TC(tile.TileContext):
    """TileContext whose final drain splits its semaphore waits across
    single-wait NOPs -- the walrus build in this image rejects CTRL
    instructions carrying 3+ sync waits ("Too many sync wait commands")."""

    def _drain_and_barrier(self, tick_clock, wait_clock):
        probe = self.nc.sync.nop(nofuse=True)
        wait_clock.add_sem_waits(probe.ins, ScopedClock({None: tick_clock.global_clock}))
        si = probe.ins.sync_info
        waits = list(si.on_wait) if si and si.on_wait else []
        if si is not None and si.on_wait:
            del si.on_wait[1:]
        for w in waits[1:]:
            nop = self.nc.sync.nop(nofuse=True)
            nsi = nop.ins.sync_info
            if nsi is None:
                nop.ins.sync_info = mybir.SyncInfo(on_wait=[w], on_update=[])
            else:
                nsi.on_wait.append(w)
        self.nc.sync.drain()
        self.nc.all_engine_barrier()
        assert self.sems is not None
        popped = self.nc._tile_sem_poison_stack.pop()
        assert popped is self._sem_poison
        self.nc.clear_and_free_semaphores(list(self.sems.allocated().values()))
        self.nc.all_engine_barrier()


_MAX_WAITS = 1


def _split_excess_waits(nc, maxw=_MAX_WAITS):
    """The walrus build in this image rejects instructions with >1 sync
    waits; spill extra waits onto NoOps inserted just before the offender
    on the same engine (all waits are preconditions, so order is free)."""
    n = 0
    for f in nc.m.functions:
        for bb in f.blocks:
            out = []
            for inst in bb.instructions:
                si = inst.sync_info
                waits = list(si.on_wait) if si and si.on_wait else []
                if len(waits) > maxw:
                    extra = waits[:-maxw]
                    del si.on_wait[: len(extra)]
                    for i in range(0, len(extra), maxw):
                        n += 1
                        nop = mybir.InstNoOp(name=f"I-wsplit-{n}-{inst.name}",
                                             ins=[], outs=[])
                        nop.engine = inst.engine
                        nop.sync_info = mybir.SyncInfo(
                            on_wait=extra[i:i + maxw], on_update=[])
                        out.append(nop)
                out.append(inst)
            bb.instructions[:] = out


def _build_nc(has_qkv_bias: bool):
    nc = bass.Bass("TRN2", target_bir_lowering=False, debug=False, num_devices=NCORES)

    xt = nc.dram_tensor("xt", [DIM, T], BF16, kind="ExternalInput")
    wqt = nc.dram_tensor("wqt", [DIM, GD], BF16, kind="ExternalInput")
    wkt = nc.dram_tensor("wkt", [DIM, GD], BF16, kind="ExternalInput")
    wvt = nc.dram_tensor("wvt", [DIM, GD], BF16, kind="ExternalInput")
    wpt = nc.dram_tensor("wpt", [GD, DIM], BF16, kind="ExternalInput")
    cosb = nc.dram_tensor("cosb", [T, HG * NFREQ], BF16, kind="ExternalInput")
    sinb = nc.dram_tensor("sinb", [T, HG * NFREQ], BF16, kind="ExternalInput")
    if has_qkv_bias:
        bq = nc.dram_tensor("bq", [GD], F32, kind="ExternalInput")
        bk = nc.dram_tensor("bk", [GD], F32, kind="ExternalInput")
        bv = nc.dram_tensor("bv", [GD], F32, kind="ExternalInput")
    out = nc.dram_tensor("out", [T, DIM], BF16, kind="ExternalOutput")

    xt_v = xt.rearrange("(do p) t -> p do t", p=P)      # [128, 16, 2048]
    wqt_v = wqt.rearrange("(do p) o -> p do o", p=P)    # [128, 16, 512]
    wkt_v = wkt.rearrange("(do p) o -> p do o", p=P)
    wvt_v = wvt.rearrange("(do p) o -> p do o", p=P)
    wpt_v = wpt.rearrange("(co p) o -> p co o", p=P)    # [128, 4, 2048]

    with _TC(nc) as tc:
        with (
            tc.tile_pool(name="const", bufs=1) as constp,
            tc.tile_pool(name="persist", bufs=1) as persist,
        ):
            identity = constp.tile([P, P], BF16)
            make_identity(nc, identity)
            ones_t = constp.tile([P, P], BF16)
            nc.vector.memset(ones_t, 1.0)
            eps_t = constp.tile([P, 1], F32)
            nc.vector.memset(eps_t, EPS)
            if has_qkv_bias:
                bias_b = constp.tile([P, 3, GD], F32)
                for bi, bten in enumerate((bq, bk, bv)):
                    bcast = bass.AP(tensor=bten.tensor, offset=bten.offset,
                                    ap=[[0, P]] + list(bten.ap))
                    nc.sync.dma_start(out=bias_b[:, bi, :], in_=bcast)

            v_sb = persist.tile([P, NT, GD], BF16)       # v, natural [t, head*128]
            rk_sb = persist.tile([P, NT, HG], F32)       # 0.12/rms(k) per (t, head)
            qt_sb = persist.tile([P, NT, HG, P], BF16)   # q^T [c, tt, h, t]
            kt_sb = persist.tile([P, NT, HG, P], BF16)
            wp_sb = persist.tile([P, HG, DIM], BF16)

            # ---------------- Phase 1: QKV + rotary + norms + transposes ------
            # K/V for t-tile i and Q for t-tile i-1 per iteration: Q trails
            # one tile so the PE starts on K/V as soon as the first weight
            # chunks land instead of waiting for all three weight matrices.
            with (
                tc.tile_pool(name="wqkv", bufs=1) as wpool,
                tc.tile_pool(name="ph1", bufs=3) as ph1,
                tc.tile_pool(name="ph1t", bufs=2) as ph1t,
                tc.tile_pool(name="pp1", bufs=1, space="PSUM") as pp1,
            ):
                wq_sb = wpool.tile([P, ND, GD], BF16)
                wk_sb = wpool.tile([P, ND, GD], BF16)
                wv_sb = wpool.tile([P, ND, GD], BF16)

                acts = {}

                def load_acts(tt):
                    xtile = ph1.tile([P, ND, P], BF16, tag="xtile",
                                     name=f"xtile{tt}")
                    nc.sync.dma_start(out=xtile,
                                      in_=xt_v[:, :, tt * P:(tt + 1) * P])
                    ctile = ph1.tile([P, HG * NFREQ], BF16, tag="ctile",
                                     name=f"ctile{tt}")
                    stile = ph1.tile([P, HG * NFREQ], BF16, tag="stile",
                                     name=f"stile{tt}")
                    nc.sync.dma_start(out=ctile, in_=cosb[tt * P:(tt + 1) * P, :])
                    nc.sync.dma_start(out=stile, in_=sinb[tt * P:(tt + 1) * P, :])
                    acts[tt] = (xtile, ctile, stile)

                # First K/V contraction needs only xtile(0) + the d=0 chunks;
                # order the queue so those land first.
                load_acts(0)
                for d in range(ND):
                    nc.sync.dma_start(out=wk_sb[:, d, :], in_=wkt_v[:, d, :])
                    nc.sync.dma_start(out=wv_sb[:, d, :], in_=wvt_v[:, d, :])
                    nc.sync.dma_start(out=wq_sb[:, d, :], in_=wqt_v[:, d, :])
                # Wp not needed until the first projection, far into phase 2.
                nc.sync.dma_start(out=wp_sb, in_=wpt_v)

                def qk_post(which, ps, tt):
                    """PSUM -> bf16 SBUF, raw-value RMS stats, in-place rotary
                    on the nonzero-frequency half of each head's channels.
                    Returns the finished [128, 512] bf16 tile."""
                    _, ctile, stile = acts[tt]
                    base = ph1t.tile([P, GD], BF16, tag=which + "base",
                                     bufs=4 if which == "q" else 2,
                                     name=f"{which}base{tt}")
                    if has_qkv_bias:
                        nc.vector.tensor_add(base, ps, bias_b[:, 1 if which == "k" else 0, :])
                    else:
                        nc.scalar.copy(out=base, in_=ps)
                    # stats on the raw values: rotation preserves row norms
                    sq = ph1t.tile([P, GD], BF16, tag="sq")
                    nc.vector.tensor_mul(sq, base, base)
                    ms = ph1t.tile([P, HG], F32, tag=which + "ms")
                    nc.vector.reduce_sum(out=ms,
                                         in_=sq.rearrange("p (h c) -> p h c", h=HG),
                                         axis=mybir.AxisListType.X)
                    rstd = ph1t.tile([P, HG], F32, tag=which + "rstd")
                    nc.scalar.activation(out=rstd, in_=ms, func=AF.Sqrt,
                                         scale=1.0 / HEAD_DIM, bias=eps_t[:, 0:1])
                    if which == "q":
                        nc.vector.reciprocal(out=rstd, in_=rstd)
                    else:
                        nc.vector.reciprocal(out=rk_sb[:, tt, :], in_=rstd)
                        nc.vector.tensor_scalar_mul(
                            rk_sb[:, tt, :], rk_sb[:, tt, :], SCALE)
                    # rotary on channels [0:32] & [64:96] of each head; the
                    # zero-frequency half is pass-through (already in base)
                    br = base.rearrange("p (h c) -> p h c", h=HG)
                    x1f = br[:, :, 0:NFREQ]
                    x2f = br[:, :, 2 * NFREQ:3 * NFREQ]
                    u1 = ph1t.tile([P, HG, NFREQ], BF16, tag="u1")
                    w2 = ph1t.tile([P, HG, NFREQ], BF16, tag="w2")
                    u2 = ph1t.tile([P, HG, NFREQ], BF16, tag="u2")
                    w1 = ph1t.tile([P, HG, NFREQ], BF16, tag="w1")
                    cr = ctile.rearrange("p (h c) -> p h c", h=HG)
                    sr = stile.rearrange("p (h c) -> p h c", h=HG)
                    nc.vector.tensor_mul(u1, x1f, cr)
                    nc.vector.tensor_mul(w2, x2f, sr)
                    nc.vector.tensor_mul(u2, x2f, cr)
                    nc.vector.tensor_mul(w1, x1f, sr)
                    nc.vector.tensor_add(x1f, u1, w2)
                    nc.vector.tensor_sub(x2f, u2, w1)
                    if which == "q":
                        for h in range(HG):
                            nc.vector.tensor_scalar_mul(
                                br[:, h, :], br[:, h, :], rstd[:, h:h + 1])
                    return base

                def qk_transpose(which, base, tt):
                    dst = qt_sb if which == "q" else kt_sb
                    for h in range(HG):
                        ptr = pp1.tile([P, P], BF16, tag="ptr", bufs=2)
                        nc.tensor.transpose(ptr, base[:, h * P:(h + 1) * P],
                                            identity)
                        if h % 2 == 0:
                            nc.scalar.copy(out=dst[:, tt, h, :], in_=ptr)
                        else:
                            nc.vector.tensor_copy(out=dst[:, tt, h, :], in_=ptr)

                # Software pipeline: per iteration i the PE runs K/V(i) then
                # Q(i-1), then transposes k(i) (whose post chain overlapped
                # the Q matmuls) and q(i-2) (chain finished an iteration ago).
                qbases = {}
                for i in range(NT + 2):
                    if 2 <= i:
                        qk_transpose("q", qbases.pop(i - 2), i - 2)
                    if i < NT:
                        if i + 1 < NT:
                            load_acts(i + 1)
                        xtile = acts[i][0]
                        ps_k = pp1.tile([P, GD], F32, tag="psk", bufs=2)
                        ps_v = pp1.tile([P, GD], F32, tag="psv", bufs=2)
                        for d in range(ND):
                            st, sp = d == 0, d == ND - 1
                            nc.tensor.matmul(ps_k, xtile[:, d, :], wk_sb[:, d, :],
                                             start=st, stop=sp)
                            nc.tensor.matmul(ps_v, xtile[:, d, :], wv_sb[:, d, :],
                                             start=st, stop=sp)
                        if has_qkv_bias:
                            nc.vector.tensor_add(v_sb[:, i, :], ps_v, bias_b[:, 2, :])
                        else:
                            nc.scalar.copy(out=v_sb[:, i, :], in_=ps_v)
                        kbase = qk_post("k", ps_k, i)
                    if 1 <= i <= NT:
                        tt = i - 1
                        xtile = acts[tt][0]
                        ps_q = pp1.tile([P, GD], F32, tag="psq", bufs=2)
                        for d in range(ND):
                            nc.tensor.matmul(ps_q, xtile[:, d, :], wq_sb[:, d, :],
                                             start=d == 0, stop=d == ND - 1)
                        qbases[tt] = qk_post("q", ps_q, tt)
                    if i < NT:
                        qk_transpose("k", kbase, i)

            # ---------------- Phase 2: attention + projection -----------------
            with (
                tc.tile_pool(name="ph2", bufs=2) as ph2,
                tc.tile_pool(name="pp2", bufs=1, space="PSUM") as pp2,
            ):
                def proj_half(ic, y_sb, half):
                    for it in (half, 2 + half):
                        for dc in range(4):
                            ps_o = pp2.tile([P, 512], F32, tag="po", bufs=2)
                            for co in range(HG):
                                nc.tensor.matmul(
                                    ps_o, y_sb[:, co, it * P:(it + 1) * P],
                                    wp_sb[:, co, dc * 512:(dc + 1) * 512],
                                    start=co == 0, stop=co == HG - 1)
                            o_sb = ph2.tile([P, 512], BF16, tag="o", bufs=3)
                            nc.vector.tensor_copy(out=o_sb, in_=ps_o)
                            nc.sync.dma_start(
                                out=out[(4 * ic + it) * P:(4 * ic + it + 1) * P,
                                        dc * 512:(dc + 1) * 512],
                                in_=o_sb)

                prev = None
                for ic in range(NI):
                    y_sb = ph2.tile([P, HG, 512], BF16, tag="y")
                    nj = 4 * (ic + 1)
                    # Heads in interleaved pairs with scores+exp prefetched one
                    # j-step ahead.  Diagonal blocks (jt >= 4*ic) only compute
                    # the surviving i >= jt columns; the single partial
                    # [128,128] block gets a triangular mask.  Half of the
                    # previous chunk's projection is emitted at each pair
                    # boundary so the PE chews on it where it would otherwise
                    # wait for the pair's PSUM accumulators to drain.
                    for hp in range(HG // 2):
                        if prev is not None:
                            proj_half(prev[0], prev[1], hp)
                        hs = (2 * hp, 2 * hp + 1)
                        ps_ys = {h: pp2.tile([P, 512], F32, tag="py", bufs=2,
                                             name=f"ps_y{h}") for h in hs}
                        ps_ls = {h: pp2.tile([P, 512], F32, tag="pl", bufs=2,
                                             name=f"ps_l{h}") for h in hs}

                        def sc_exp(jt, h):
                            r = jt - 4 * ic
                            off = max(0, r) * P
                            ps_s = pp2.tile([P, 512], F32, tag="ps", bufs=2)
                            nc.tensor.matmul(
                                ps_s[:, off:], kt_sb[:, jt, h, :],
                                qt_sb[:, 4 * ic + max(0, r):4 * (ic + 1), h, :],
                                start=True, stop=True)
                            p_sb = ph2.tile([P, 512], BF16, tag="p", bufs=6)
                            nc.scalar.activation(
                                out=p_sb[:, off:], in_=ps_s[:, off:], func=AF.Exp,
                                scale=rk_sb[:, jt, h:h + 1])
                            if r >= 0:
                                nc.gpsimd.affine_select(
                                    out=p_sb[:, off:off + P],
                                    in_=p_sb[:, off:off + P],
                                    pattern=[[1, P]], channel_multiplier=-1,
                                    base=0,
                                    compare_op=mybir.AluOpType.is_ge, fill=0.0)
                            return p_sb, off

                        ps = {h: sc_exp(0, h) for h in hs}
                        for jt in range(nj):
                            nxt = {}
                            if jt + 1 < nj:
                                nxt = {h: sc_exp(jt + 1, h) for h in hs}
                            for h in hs:
                                st, sp = jt == 0, jt == nj - 1
                                p_sb, off = ps[h]
                                nc.tensor.matmul(ps_ls[h][:, off:], ones_t,
                                                 p_sb[:, off:],
                                                 start=st, stop=sp)
                                nc.tensor.matmul(
                                    ps_ys[h][:, off:],
                                    v_sb[:, jt, h * P:(h + 1) * P],
                                    p_sb[:, off:], start=st, stop=sp)
                            ps = nxt
                        for h in hs:
                            linv = ph2.tile([P, 512], F32, tag="linv")
                            nc.vector.reciprocal(out=linv, in_=ps_ls[h])
                            nc.vector.tensor_mul(y_sb[:, h, :], ps_ys[h], linv)
                    prev = (ic, y_sb)
                proj_half(prev[0], prev[1], 0)
                proj_half(prev[0], prev[1], 1)
    _split_excess_waits(nc)
    return nc


_NC_CACHE = {}
_RUN_KWARGS = {}      # test harness hook: e.g. {"trace": True}
_LAST_RESULT = None   # BassKernelResults of the most recent run


def _rotary_tables():
    freq = (1.0 / 1024.0) ** np.linspace(0.0, 1.0, NFREQ, dtype=np.float32)
    theta = np.arange(T, dtype=np.float32)[:, None] * freq[None, :]      # [T, 32]
    cos = np.cos(theta).astype(np.float32)
    sin = np.sin(theta).astype(np.float32)
    cosb = np.tile(cos, (1, HG)).astype(ml_dtypes.bfloat16)              # [T, 128]
    sinb = np.tile(sin, (1, HG)).astype(ml_dtypes.bfloat16)
    return np.ascontiguousarray(cosb), np.ascontiguousarray(sinb)


def kernel(x, Wq, bq, Wk, bk, Wv, bv, Wp, bp):
    x = np.asarray(x, np.float32)
    Wq, Wk, Wv, Wp = (np.asarray(a, np.float32) for a in (Wq, Wk, Wv, Wp))
    bq, bk, bv, bp = (np.asarray(a, np.float32) for a in (bq, bk, bv, bp))

    has_bias = bool(np.any(bq) or np.any(bk) or np.any(bv))
    if has_bias not in _NC_CACHE:
        _NC_CACHE[has_bias] = _build_nc(has_bias)
    nc = _NC_CACHE[has_bias]

    def b16(a):
        return np.ascontiguousarray(a.astype(ml_dtypes.bfloat16))

    cosb, sinb = _rotary_tables()
    in_maps = []
    for c in range(NCORES):
        b, g = divmod(c, NCORES // B)
        sl = slice(g * GD, (g + 1) * GD)
        m = {
            "xt": b16(x[b].T),
            "wqt": b16(Wq[sl, :].T),
            "wkt": b16(Wk[sl, :].T),
            "wvt": b16(Wv[sl, :].T),
            "wpt": b16(Wp[:, sl].T),
            "cosb": cosb,
            "sinb": sinb,
        }
        if has_bias:
            m["bq"] = np.ascontiguousarray(bq[sl])
            m["bk"] = np.ascontiguousarray(bk[sl])
            m["bv"] = np.ascontiguousarray(bv[sl])
        in_maps.append(m)

    res = run_bass_kernel_spmd(nc, in_maps, list(range(NCORES)), **_RUN_KWARGS)
    global _LAST_RESULT
    _LAST_RESULT = res
    out = np.zeros((B, T, DIM), np.float32)
    for c in range(NCORES):
        out[c // (NCORES // B)] += res.results[c]["out"].astype(np.float32)
    out += bp[None, None, :]
    return out


# revision 8
# speedup vs baseline: 1.2006x; 1.0570x over previous
"""Trainium2 Bass kernel for causal self-attention with QK RMS-norm + rotary.

Full (unsharded) inputs in, full output out.  Internally sharded over 8
NeuronCores: data parallel on batch (2) x tensor parallel on head groups
(16 heads -> 4 groups of 4).  Each core computes q/k/v for its 4 heads on
its batch, causal flash-style attention, and a partial output projection
(its 512-column slice of Wp's input dim); the host sums the 4 partials per
batch ("all-reduce after proj" done host-side) and adds the output bias.

All matmul operands are bf16 (fp32 PSUM accumulation): same PE rate as
fp32r but with fast weight loads, half the DMA/SBUF traffic, and 2x DVE
throughput on SBUF-resident elementwise work.  Numerics validated against
the fp32 reference at ~3e-3 max-rel-err (gate 2e-2).

Per-core pipeline (single Bass program, SPMD over 8 cores):
  Phase 1, per 128-row t-tile: QKV projections with x^T tiles stationary
    (q/k/v share each weight load); ScalarE evacuates PSUM to bf16 SBUF;
    RMS stats are taken on the *raw* q/k (rotation preserves row norms) so
    they overlap the rotary; rotary touches only the 64 nonzero-frequency
    channels per head (the other half is pass-through, already in place
    from the copy); q scaled by 1/rms in place; k's norm is folded into
    the softmax exp scale (0.12/rms(k_j) per scores^T partition); q^T/k^T
    built with bf16 PE transposes and kept in SBUF (no DRAM spill).
  Phase 2, per 512-column query chunk, per head: scores^T = k^T-tile @ q^T
    so the softmax denominator comes from a ones-stationary matmul and
    attn@v needs no transpose of p; exp on ScalarE (bf16 out); causal
    diagonal blocks run at reduced moving size (only j<=i columns) with a
    GpSimd triangular affine_select on the single partial [128,128] block;
    y^T and l accumulate in PSUM over j-tiles; 1/l via the fast DVE
    reciprocal; output projection contracts the 4 head-slices of y^T
    against Wp^T, interleaved with the next chunk's attention; partial
    outputs written bf16 and reduced host-side.
"""

import os
import sys

import numpy as np

try:
    import concourse.bass as bass
except ImportError:  # fall back to the repo checkout baked into the image
    for _p in ("/opt/trn_rl_repo", "/root/.axon_site/_ro/trn_rl_repo"):
        if os.path.isdir(_p) and _p not in sys.path:
            sys.path.append(_p)
    import concourse.bass as bass

import ml_dtypes

import concourse.mybir as mybir
import concourse.tile as tile
from concourse.bass_utils import run_bass_kernel_spmd
from concourse.masks import make_identity
from concourse.vector_clock import ScopedClock

F32 = mybir.dt.float32
BF16 = mybir.dt.bfloat16
AF = mybir.ActivationFunctionType

DIM = 2048
HEAD_DIM = 128
NUM_HEADS = 16
B, T = 2, 2048
EPS = 1.1920929e-07
SCALE = 0.12

NCORES = 8
HG = 4                    # heads per core
GD = HG * HEAD_DIM        # 512: per-core q/k/v width and Wp input slice
NT = T // 128             # 16 t-tiles
ND = DIM // 128           # 16 contraction tiles
NI = T // 512             # 4 query chunks
P = 128
NFREQ = HEAD_DIM // 4     # 32 nonzero-frequency channels per head


class _TC(tile.TileContext):
    """TileContext whose final drain splits its semaphore waits across
    single-wait NOPs -- the walrus build in this image rejects CTRL
    instructions carrying 3+ sync waits ("Too many sync wait commands")."""

    def _drain_and_barrier(self, tick_clock, wait_clock):
        probe = self.nc.sync.nop(nofuse=True)
        wait_clock.add_sem_waits(probe.ins, ScopedClock({None: tick_clock.global_clock}))
        si = probe.ins.sync_info
        waits = list(si.on_wait) if si and si.on_wait else []
        if si is not None and si.on_wait:
            del si.on_wait[1:]
        for w in waits[1:]:
            nop = self.nc.sync.nop(nofuse=True)
            nsi = nop.ins.sync_info
            if nsi is None:
                nop.ins.sync_info = mybir.SyncInfo(on_wait=[w], on_update=[])
            else:
                nsi.on_wait.append(w)
        self.nc.sync.drain()
        self.nc.all_engine_barrier()
        assert self.sems is not None
        popped = self.nc._tile_sem_poison_stack.pop()
        assert popped is self._sem_poison
        self.nc.clear_and_free_semaphores(list(self.sems.allocated().values()))
        self.nc.all_engine_barrier()


_MAX_WAITS = 1


def _split_excess_waits(nc, maxw=_MAX_WAITS):
    """The walrus build in this image rejects instructions with >1 sync
    waits; spill extra waits onto NoOps inserted just before the offender
    on the same engine (all waits are preconditions, so order is free)."""
    n = 0
    for f in nc.m.functions:
        for bb in f.blocks:
            out = []
            for inst in bb.instructions:
                si = inst.sync_info
                waits = list(si.on_wait) if si and si.on_wait else []
                if len(waits) > maxw:
                    extra = waits[:-maxw]
                    del si.on_wait[: len(extra)]
                    for i in range(0, len(extra), maxw):
                        n += 1
                        nop = mybir.InstNoOp(name=f"I-wsplit-{n}-{inst.name}",
                                             ins=[], outs=[])
                        nop.engine = inst.engine
                        nop.sync_info = mybir.SyncInfo(
                            on_wait=extra[i:i + maxw], on_update=[])
                        out.append(nop)
                out.append(inst)
            bb.instructions[:] = out


def _build_nc(has_qkv_bias: bool):
    nc = bass.Bass("TRN2", target_bir_lowering=False, debug=False, num_devices=NCORES)

    xt = nc.dram_tensor("xt", [DIM, T], BF16, kind="ExternalInput")
    wqt = nc.dram_tensor("wqt", [DIM, GD], BF16, kind="ExternalInput")
    wkt = nc.dram_tensor("wkt", [DIM, GD], BF16, kind="ExternalInput")
    wvt = nc.dram_tensor("wvt", [DIM, GD], BF16, kind="ExternalInput")
    wpt = nc.dram_tensor("wpt", [GD, DIM], BF16, kind="ExternalInput")
    cosb = nc.dram_tensor("cosb", [T, HG * NFREQ], BF16, kind="ExternalInput")
    sinb = nc.dram_tensor("sinb", [T, HG * NFREQ], BF16, kind="ExternalInput")
    if has_qkv_bias:
        bq = nc.dram_tensor("bq", [GD], F32, kind="ExternalInput")
        bk = nc.dram_tensor("bk", [GD], F32, kind="ExternalInput")
        bv = nc.dram_tensor("bv", [GD], F32, kind="ExternalInput")
    out = nc.dram_tensor("out", [T, DIM], BF16, kind="ExternalOutput")

    xt_v = xt.rearrange("(do p) t -> p do t", p=P)      # [128, 16, 2048]
    wqt_v = wqt.rearrange("(do p) o -> p do o", p=P)    # [128, 16, 512]
    wkt_v = wkt.rearrange("(do p) o -> p do o", p=P)
    wvt_v = wvt.rearrange("(do p) o -> p do o", p=P)
    wpt_v = wpt.rearrange("(co p) o -> p co o", p=P)    # [128, 4, 2048]

    with _TC(nc) as tc:
        with (
            tc.tile_pool(name="const", bufs=1) as constp,
            tc.tile_pool(name="persist", bufs=1) as persist,
        ):
            identity = constp.tile([P, P], BF16)
            make_identity(nc, identity)
            ones_t = constp.tile([P, P], BF16)
            nc.vector.memset(ones_t, 1.0)
            eps_t = constp.tile([P, 1], F32)
            nc.vector.memset(eps_t, EPS)
            if has_qkv_bias:
                bias_b = constp.tile([P, 3, GD], F32)
                for bi, bten in enumerate((bq, bk, bv)):
                    bcast = bass.AP(tensor=bten.tensor, offset=bten.offset,
                                    ap=[[0, P]] + list(bten.ap))
                    nc.sync.dma_start(out=bias_b[:, bi, :], in_=bcast)

            v_sb = persist.tile([P, NT, GD], BF16)       # v, natural [t, head*128]
            rk_sb = persist.tile([P, NT, HG], F32)       # 0.12/rms(k) per (t, head)
            qt_sb = persist.tile([P, NT, HG, P], BF16)   # q^T [c, tt, h, t]
            kt_sb = persist.tile([P, NT, HG, P], BF16)
            wp_sb = persist.tile([P, HG, DIM], BF16)

            # ---------------- Phase 1: QKV + rotary + norms + transposes ------
            # K/V for t-tile i and Q for t-tile i-1 per iteration: Q trails
            # one tile so the PE starts on K/V as soon as the first weight
            # chunks land instead of waiting for all three weight matrices.
            with (
                tc.tile_pool(name="wqkv", bufs=1) as wpool,
                tc.tile_pool(name="ph1", bufs=3) as ph1,
                tc.tile_pool(name="ph1t", bufs=2) as ph1t,
                tc.tile_pool(name="pp1", bufs=1, space="PSUM") as pp1,
            ):
                wq_sb = wpool.tile([P, ND, GD], BF16)
                wk_sb = wpool.tile([P, ND, GD], BF16)
                wv_sb = wpool.tile([P, ND, GD], BF16)

                acts = {}

                def load_acts(tt):
                    xtile = ph1.tile([P, ND, P], BF16, tag="xtile",
                                     name=f"xtile{tt}")
                    nc.gpsimd.dma_start(out=xtile,
                                        in_=xt_v[:, :, tt * P:(tt + 1) * P])
                    ctile = ph1.tile([P, HG * NFREQ], BF16, tag="ctile",
                                     name=f"ctile{tt}")
                    stile = ph1.tile([P, HG * NFREQ], BF16, tag="stile",
                                     name=f"stile{tt}")
                    nc.gpsimd.dma_start(out=ctile, in_=cosb[tt * P:(tt + 1) * P, :])
                    nc.gpsimd.dma_start(out=stile, in_=sinb[tt * P:(tt + 1) * P, :])
                    acts[tt] = (xtile, ctile, stile)

                # Four DMA queues in parallel so no ordering stall: wk on
                # sync, wv on vector, wq on scalar, activations on gpsimd.
                load_acts(0)
                for d in range(ND):
                    nc.sync.dma_start(out=wk_sb[:, d, :], in_=wkt_v[:, d, :])
                    nc.sync.dma_start(out=wv_sb[:, d, :], in_=wvt_v[:, d, :])
                    nc.scalar.dma_start(out=wq_sb[:, d, :], in_=wqt_v[:, d, :])
                # Wp not needed until the first projection, far into phase 2.
                nc.sync.dma_start(out=wp_sb, in_=wpt_v)

                def qk_post(which, ps, tt):
                    """PSUM -> bf16 SBUF, raw-value RMS stats, in-place rotary
                    on the nonzero-frequency half of each head's channels.
                    Returns the finished [128, 512] bf16 tile."""
                    _, ctile, stile = acts[tt]
                    base = ph1t.tile([P, GD], BF16, tag=which + "base",
                                     bufs=4 if which == "q" else 2,
                                     name=f"{which}base{tt}")
                    if has_qkv_bias:
                        nc.vector.tensor_add(base, ps, bias_b[:, 1 if which == "k" else 0, :])
                    else:
                        nc.scalar.copy(out=base, in_=ps)
                    # stats on the raw values: rotation preserves row norms
                    sq = ph1t.tile([P, GD], BF16, tag="sq")
                    nc.vector.tensor_mul(sq, base, base)
                    ms = ph1t.tile([P, HG], F32, tag=which + "ms")
                    nc.vector.reduce_sum(out=ms,
                                         in_=sq.rearrange("p (h c) -> p h c", h=HG),
                                         axis=mybir.AxisListType.X)
                    rstd = ph1t.tile([P, HG], F32, tag=which + "rstd")
                    nc.scalar.activation(out=rstd, in_=ms, func=AF.Sqrt,
                                         scale=1.0 / HEAD_DIM, bias=eps_t[:, 0:1])
                    if which == "q":
                        nc.vector.reciprocal(out=rstd, in_=rstd)
                    else:
                        nc.vector.reciprocal(out=rk_sb[:, tt, :], in_=rstd)
                        nc.vector.tensor_scalar_mul(
                            rk_sb[:, tt, :], rk_sb[:, tt, :], SCALE)
                    # rotary on channels [0:32] & [64:96] of each head; the
                    # zero-frequency half is pass-through (already in base)
                    br = base.rearrange("p (h c) -> p h c", h=HG)
                    x1f = br[:, :, 0:NFREQ]
                    x2f = br[:, :, 2 * NFREQ:3 * NFREQ]
                    u1 = ph1t.tile([P, HG, NFREQ], BF16, tag="u1")
                    w2 = ph1t.tile([P, HG, NFREQ], BF16, tag="w2")
                    u2 = ph1t.tile([P, HG, NFREQ], BF16, tag="u2")
                    w1 = ph1t.tile([P, HG, NFREQ], BF16, tag="w1")
                    cr = ctile.rearrange("p (h c) -> p h c", h=HG)
                    sr = stile.rearrange("p (h c) -> p h c", h=HG)
                    nc.vector.tensor_mul(u1, x1f, cr)
                    nc.vector.tensor_mul(w2, x2f, sr)
                    nc.vector.tensor_mul(u2, x2f, cr)
                    nc.vector.tensor_mul(w1, x1f, sr)
                    nc.vector.tensor_add(x1f, u1, w2)
                    nc.vector.tensor_sub(x2f, u2, w1)
                    if which == "q":
                        for h in range(HG):
                            nc.vector.tensor_scalar_mul(
                                br[:, h, :], br[:, h, :], rstd[:, h:h + 1])
                    return base

                def qk_transpose(which, base, tt):
                    dst = qt_sb if which == "q" else kt_sb
                    for h in range(HG):
                        ptr = pp1.tile([P, P], BF16, tag="ptr", bufs=2)
                        nc.tensor.transpose(ptr, base[:, h * P:(h + 1) * P],
                                            identity)
                        if h % 2 == 0:
                            nc.scalar.copy(out=dst[:, tt, h, :], in_=ptr)
                        else:
                            nc.vector.tensor_copy(out=dst[:, tt, h, :], in_=ptr)

                # Software pipeline: per iteration i the PE runs K/V(i) then
                # Q(i-1), then transposes k(i) (whose post chain overlapped
                # the Q matmuls) and q(i-2) (chain finished an iteration ago).
                qbases = {}
                for i in range(NT + 2):
                    if 2 <= i:
                        qk_transpose("q", qbases.pop(i - 2), i - 2)
                    if i < NT:
                        if i + 1 < NT:
                            load_acts(i + 1)
                        xtile = acts[i][0]
                        ps_k = pp1.tile([P, GD], F32, tag="psk", bufs=2)
                        ps_v = pp1.tile([P, GD], F32, tag="psv", bufs=2)
                        for d in range(ND):
                            st, sp = d == 0, d == ND - 1
                            nc.tensor.matmul(ps_k, xtile[:, d, :], wk_sb[:, d, :],
                                             start=st, stop=sp)
                            nc.tensor.matmul(ps_v, xtile[:, d, :], wv_sb[:, d, :],
                                             start=st, stop=sp)
                        if has_qkv_bias:
                            nc.vector.tensor_add(v_sb[:, i, :], ps_v, bias_b[:, 2, :])
                        else:
                            nc.scalar.copy(out=v_sb[:, i, :], in_=ps_v)
                        kbase = qk_post("k", ps_k, i)
                    if 1 <= i <= NT:
                        tt = i - 1
                        xtile = acts[tt][0]
                        ps_q = pp1.tile([P, GD], F32, tag="psq", bufs=2)
                        for d in range(ND):
                            nc.tensor.matmul(ps_q, xtile[:, d, :], wq_sb[:, d, :],
                                             start=d == 0, stop=d == ND - 1)
                        qbases[tt] = qk_post("q", ps_q, tt)
                    if i < NT:
                        qk_transpose("k", kbase, i)

            # ---------------- Phase 2: attention + projection -----------------
            with (
                tc.tile_pool(name="ph2", bufs=2) as ph2,
                tc.tile_pool(name="pp2", bufs=1, space="PSUM") as pp2,
            ):
                def proj_groups(ic, y_sb):
                    """16 closures, each emitting one 512-col output block:
                    4 accumulating matmuls + a DVE evacuation + the DMA out.
                    Paced into the next chunk's attention loop so po banks
                    never stall the PE and the output DMA spreads out."""
                    def mk(it, dc):
                        def emit():
                            ps_o = pp2.tile([P, 512], F32, tag="po", bufs=2)
                            for co in range(HG):
                                nc.tensor.matmul(
                                    ps_o, y_sb[:, co, it * P:(it + 1) * P],
                                    wp_sb[:, co, dc * 512:(dc + 1) * 512],
                                    start=co == 0, stop=co == HG - 1)
                            o_sb = ph2.tile([P, 512], BF16, tag="o", bufs=3)
                            nc.vector.tensor_copy(out=o_sb, in_=ps_o)
                            nc.sync.dma_start(
                                out=out[(4 * ic + it) * P:(4 * ic + it + 1) * P,
                                        dc * 512:(dc + 1) * 512],
                                in_=o_sb)
                        return emit
                    return [mk(it, dc) for it in range(4) for dc in range(4)]

                pending = []
                prev_y = None
                for ic in range(NI):
                    if prev_y is not None:
                        pending = proj_groups(ic - 1, prev_y)
                    y_sb = ph2.tile([P, HG, 512], BF16, tag="y")
                    nj = 4 * (ic + 1)
                    steps_left = 2 * nj
                    # Heads in interleaved pairs with scores+exp prefetched one
                    # j-step ahead.  Diagonal blocks (jt >= 4*ic) only compute
                    # the surviving i >= jt columns; the single partial
                    # [128,128] block gets a triangular mask.  The previous
                    # chunk's projection blocks are paced into the j-loop.
                    for hp in range(HG // 2):
                        hs = (2 * hp, 2 * hp + 1)
                        ps_ys = {h: pp2.tile([P, 512], F32, tag="py", bufs=2,
                                             name=f"ps_y{h}") for h in hs}
                        ps_ls = {h: pp2.tile([P, 512], F32, tag="pl", bufs=2,
                                             name=f"ps_l{h}") for h in hs}

                        def sc_exp(jt, h):
                            r = jt - 4 * ic
                            off = max(0, r) * P
                            ps_s = pp2.tile([P, 512], F32, tag="ps", bufs=2)
                            nc.tensor.matmul(
                                ps_s[:, off:], kt_sb[:, jt, h, :],
                                qt_sb[:, 4 * ic + max(0, r):4 * (ic + 1), h, :],
                                start=True, stop=True)
                            p_sb = ph2.tile([P, 512], BF16, tag="p", bufs=6)
                            nc.scalar.activation(
                                out=p_sb[:, off:], in_=ps_s[:, off:], func=AF.Exp,
                                scale=rk_sb[:, jt, h:h + 1])
                            if r >= 0:
                                nc.gpsimd.affine_select(
                                    out=p_sb[:, off:off + P],
                                    in_=p_sb[:, off:off + P],
                                    pattern=[[1, P]], channel_multiplier=-1,
                                    base=0,
                                    compare_op=mybir.AluOpType.is_ge, fill=0.0)
                            return p_sb, off

                        ps = {h: sc_exp(0, h) for h in hs}
                        for jt in range(nj):
                            nxt = {}
                            if jt + 1 < nj:
                                nxt = {h: sc_exp(jt + 1, h) for h in hs}
                            for h in hs:
                                st, sp = jt == 0, jt == nj - 1
                                p_sb, off = ps[h]
                                nc.tensor.matmul(ps_ls[h][:, off:], ones_t,
                                                 p_sb[:, off:],
                                                 start=st, stop=sp)
                                nc.tensor.matmul(
                                    ps_ys[h][:, off:],
                                    v_sb[:, jt, h * P:(h + 1) * P],
                                    p_sb[:, off:], start=st, stop=sp)
                            ps = nxt
                            # pace the previous chunk's projection through
                            n_emit = -(-len(pending) // steps_left)
                            steps_left -= 1
                            for _ in range(n_emit):
                                pending.pop(0)()
                        # Fast PSUM evacuation first (ACT takes l, DVE takes
                        # y) so the next pair's accumulators free quickly;
                        # the reciprocal+normalize chain runs behind on DVE.
                        l_sbs, y_raws = {}, {}
                        for h in hs:
                            l_sbs[h] = ph2.tile([P, 512], BF16, tag="l_sb",
                                                name=f"l_sb{h}")
                            nc.scalar.copy(out=l_sbs[h], in_=ps_ls[h])
                            y_raws[h] = ph2.tile([P, 512], BF16, tag="y_raw",
                                                 name=f"y_raw{h}")
                            nc.vector.tensor_copy(out=y_raws[h], in_=ps_ys[h])
                        with nc.allow_low_precision(reason="2e-2 tolerance"):
                            for h in hs:
                                linv = ph2.tile([P, 512], BF16, tag="linv")
                                nc.vector.reciprocal(out=linv, in_=l_sbs[h])
                                nc.vector.tensor_mul(y_sb[:, h, :], y_raws[h],
                                                     linv)
                    prev_y = y_sb
                for emit in pending:
                    emit()
                for emit in proj_groups(NI - 1, prev_y):
                    emit()
    _split_excess_waits(nc)
    return nc


_NC_CACHE = {}
_RUN_KWARGS = {}      # test harness hook: e.g. {"trace": True}
_LAST_RESULT = None   # BassKernelResults of the most recent run


def _rotary_tables():
    freq = (1.0 / 1024.0) ** np.linspace(0.0, 1.0, NFREQ, dtype=np.float32)
    theta = np.arange(T, dtype=np.float32)[:, None] * freq[None, :]      # [T, 32]
    cos = np.cos(theta).astype(np.float32)
    sin = np.sin(theta).astype(np.float32)
    cosb = np.tile(cos, (1, HG)).astype(ml_dtypes.bfloat16)              # [T, 128]
    sinb = np.tile(sin, (1, HG)).astype(ml_dtypes.bfloat16)
    return np.ascontiguousarray(cosb), np.ascontiguousarray(sinb)


def kernel(x, Wq, bq, Wk, bk, Wv, bv, Wp, bp):
    x = np.asarray(x, np.float32)
    Wq, Wk, Wv, Wp = (np.asarray(a, np.float32) for a in (Wq, Wk, Wv, Wp))
    bq, bk, bv, bp = (np.asarray(a, np.float32) for a in (bq, bk, bv, bp))

    has_bias = bool(np.any(bq) or np.any(bk) or np.any(bv))
    if has_bias not in _NC_CACHE:
        _NC_CACHE[has_bias] = _build_nc(has_bias)
    nc = _NC_CACHE[has_bias]

    def b16(a):
        return np.ascontiguousarray(a.astype(ml_dtypes.bfloat16))

    cosb, sinb = _rotary_tables()
    in_maps = []
    for c in range(NCORES):
        b, g = divmod(c, NCORES // B)
        sl = slice(g * GD, (g + 1) * GD)
        m = {
            "xt": b16(x[b].T),
            "wqt": b16(Wq[sl, :].T),
            "wkt": b16(Wk[sl, :].T),
            "wvt": b16(Wv[sl, :].T),
            "wpt": b16(Wp[:, sl].T),
            "cosb": cosb,
            "sinb": sinb,
        }
        if has_bias:
            m["bq"] = np.ascontiguousarray(bq[sl])
            m["bk"] = np.ascontiguousarray(bk[sl])
            m["bv"] = np.ascontiguousarray(bv[sl])
        in_maps.append(m)

    res = run_bass_kernel_spmd(nc, in_maps, list(range(NCORES)), **_RUN_KWARGS)
    global _LAST_RESULT
    _LAST_RESULT = res
    out = np.zeros((B, T, DIM), np.float32)
    for c in range(NCORES):
        out[c // (NCORES // B)] += res.results[c]["out"].astype(np.float32)
    out += bp[None, None, :]
    return out


# revision 13
# speedup vs baseline: 1.2297x; 1.0242x over previous
"""Trainium2 Bass kernel for causal self-attention with QK RMS-norm + rotary.

Full (unsharded) inputs in, full output out.  Internally sharded over 8
NeuronCores: data parallel on batch (2) x tensor parallel on head groups
(16 heads -> 4 groups of 4).  Each core computes q/k/v for its 4 heads on
its batch, causal flash-style attention, and a partial output projection
(its 512-column slice of Wp's input dim); the host sums the 4 partials per
batch ("all-reduce after proj" done host-side) and adds the output bias.

All matmul operands are bf16 (fp32 PSUM accumulation): same PE rate as
fp32r but with fast weight loads, half the DMA/SBUF traffic, and 2x DVE
throughput on SBUF-resident elementwise work.  Numerics validated against
the fp32 reference at ~3e-3 max-rel-err (gate 2e-2).

Per-core pipeline (single Bass program, SPMD over 8 cores):
  Phase 1, per 128-row t-tile: QKV projections with x^T tiles stationary
    (q/k/v share each weight load); ScalarE evacuates PSUM to bf16 SBUF;
    RMS stats are taken on the *raw* q/k (rotation preserves row norms) so
    they overlap the rotary; rotary touches only the 64 nonzero-frequency
    channels per head (the other half is pass-through, already in place
    from the copy); q scaled by 1/rms in place; k's norm is folded into
    the softmax exp scale (0.12/rms(k_j) per scores^T partition); q^T/k^T
    built with bf16 PE transposes and kept in SBUF (no DRAM spill).
  Phase 2, per 512-column query chunk, per head: scores^T = k^T-tile @ q^T
    so the softmax denominator comes from a ones-stationary matmul and
    attn@v needs no transpose of p; exp on ScalarE (bf16 out); causal
    diagonal blocks run at reduced moving size (only j<=i columns) with a
    GpSimd triangular affine_select on the single partial [128,128] block;
    y^T and l accumulate in PSUM over j-tiles; 1/l via the fast DVE
    reciprocal; output projection contracts the 4 head-slices of y^T
    against Wp^T, interleaved with the next chunk's attention; partial
    outputs written bf16 and reduced host-side.
"""

import os
import sys

import numpy as np

try:
    import concourse.bass as bass
except ImportError:  # fall back to the repo checkout baked into the image
    for _p in ("/opt/trn_rl_repo", "/root/.axon_site/_ro/trn_rl_repo"):
        if os.path.isdir(_p) and _p not in sys.path:
            sys.path.append(_p)
    import concourse.bass as bass

import ml_dtypes

import concourse.mybir as mybir
import concourse.tile as tile
from concourse.bass_utils import run_bass_kernel_spmd
from concourse.masks import make_identity
from concourse.vector_clock import ScopedClock

F32 = mybir.dt.float32
BF16 = mybir.dt.bfloat16
AF = mybir.ActivationFunctionType

DIM = 2048
HEAD_DIM = 128
NUM_HEADS = 16
B, T = 2, 2048
EPS = 1.1920929e-07
SCALE = 0.12

NCORES = 8
HG = 4                    # heads per core
GD = HG * HEAD_DIM        # 512: per-core q/k/v width and Wp input slice
NT = T // 128             # 16 t-tiles
ND = DIM // 128           # 16 contraction tiles
NI = T // 512             # 4 query chunks
P = 128
NFREQ = HEAD_DIM // 4     # 32 nonzero-frequency channels per head


class _TC(tile.TileContext):
    """TileContext whose final drain splits its semaphore waits across
    single-wait NOPs -- the walrus build in this image rejects CTRL
    instructions carrying 3+ sync waits ("Too many sync wait commands")."""

    def _drain_and_barrier(self, tick_clock, wait_clock):
        probe = self.nc.sync.nop(nofuse=True)
        wait_clock.add_sem_waits(probe.ins, ScopedClock({None: tick_clock.global_clock}))
        si = probe.ins.sync_info
        waits = list(si.on_wait) if si and si.on_wait else []
        if si is not None and si.on_wait:
            del si.on_wait[1:]
        for w in waits[1:]:
            nop = self.nc.sync.nop(nofuse=True)
            nsi = nop.ins.sync_info
            if nsi is None:
                nop.ins.sync_info = mybir.SyncInfo(on_wait=[w], on_update=[])
            else:
                nsi.on_wait.append(w)
        self.nc.sync.drain()
        self.nc.all_engine_barrier()
        assert self.sems is not None
        popped = self.nc._tile_sem_poison_stack.pop()
        assert popped is self._sem_poison
        self.nc.clear_and_free_semaphores(list(self.sems.allocated().values()))
        self.nc.all_engine_barrier()


_MAX_WAITS = 1


def _split_excess_waits(nc, maxw=_MAX_WAITS):
    """The walrus build in this image rejects instructions with >1 sync
    waits; spill extra waits onto NoOps inserted just before the offender
    on the same engine (all waits are preconditions, so order is free)."""
    n = 0
    for f in nc.m.functions:
        for bb in f.blocks:
            out = []
            for inst in bb.instructions:
                si = inst.sync_info
                waits = list(si.on_wait) if si and si.on_wait else []
                if len(waits) > maxw:
                    extra = waits[:-maxw]
                    del si.on_wait[: len(extra)]
                    for i in range(0, len(extra), maxw):
                        n += 1
                        nop = mybir.InstNoOp(name=f"I-wsplit-{n}-{inst.name}",
                                             ins=[], outs=[])
                        nop.engine = inst.engine
                        nop.sync_info = mybir.SyncInfo(
                            on_wait=extra[i:i + maxw], on_update=[])
                        out.append(nop)
                out.append(inst)
            bb.instructions[:] = out


def _build_nc(has_qkv_bias: bool):
    nc = bass.Bass("TRN2", target_bir_lowering=False, debug=False, num_devices=NCORES)

    xt = nc.dram_tensor("xt", [DIM, T], BF16, kind="ExternalInput")
    wqt = nc.dram_tensor("wqt", [DIM, GD], BF16, kind="ExternalInput")
    wkt = nc.dram_tensor("wkt", [DIM, GD], BF16, kind="ExternalInput")
    wvt = nc.dram_tensor("wvt", [DIM, GD], BF16, kind="ExternalInput")
    wpt = nc.dram_tensor("wpt", [GD, DIM], BF16, kind="ExternalInput")
    cosb = nc.dram_tensor("cosb", [T, HG * NFREQ], BF16, kind="ExternalInput")
    sinb = nc.dram_tensor("sinb", [T, HG * NFREQ], BF16, kind="ExternalInput")
    if has_qkv_bias:
        bq = nc.dram_tensor("bq", [GD], F32, kind="ExternalInput")
        bk = nc.dram_tensor("bk", [GD], F32, kind="ExternalInput")
        bv = nc.dram_tensor("bv", [GD], F32, kind="ExternalInput")
    out = nc.dram_tensor("out", [T, DIM], BF16, kind="ExternalOutput")

    xt_v = xt.rearrange("(do p) t -> p do t", p=P)      # [128, 16, 2048]
    wqt_v = wqt.rearrange("(do p) o -> p do o", p=P)    # [128, 16, 512]
    wkt_v = wkt.rearrange("(do p) o -> p do o", p=P)
    wvt_v = wvt.rearrange("(do p) o -> p do o", p=P)
    wpt_v = wpt.rearrange("(co p) o -> p co o", p=P)    # [128, 4, 2048]

    with _TC(nc) as tc:
        with (
            tc.tile_pool(name="const", bufs=1) as constp,
            tc.tile_pool(name="persist", bufs=1) as persist,
        ):
            identity = constp.tile([P, P], BF16)
            make_identity(nc, identity)
            ones_t = constp.tile([P, P], BF16)
            nc.vector.memset(ones_t, 1.0)
            eps_t = constp.tile([P, 1], F32)
            nc.vector.memset(eps_t, EPS)
            if has_qkv_bias:
                bias_b = constp.tile([P, 3, GD], F32)
                for bi, bten in enumerate((bq, bk, bv)):
                    bcast = bass.AP(tensor=bten.tensor, offset=bten.offset,
                                    ap=[[0, P]] + list(bten.ap))
                    nc.sync.dma_start(out=bias_b[:, bi, :], in_=bcast)

            v_sb = persist.tile([P, NT, GD], BF16)       # v, natural [t, head*128]
            rk_sb = persist.tile([P, NT, HG], F32)       # 0.12/rms(k) per (t, head)
            qt_sb = persist.tile([P, NT, HG, P], BF16)   # q^T [c, tt, h, t]
            kt_sb = persist.tile([P, NT, HG, P], BF16)
            wp_sb = persist.tile([P, HG, DIM], BF16)

            # ---------------- Phase 1: QKV + rotary + norms + transposes ------
            # K/V for t-tile i and Q for t-tile i-1 per iteration: Q trails
            # one tile so the PE starts on K/V as soon as the first weight
            # chunks land instead of waiting for all three weight matrices.
            with (
                tc.tile_pool(name="wqkv", bufs=1) as wpool,
                tc.tile_pool(name="ph1", bufs=3) as ph1,
                tc.tile_pool(name="ph1t", bufs=2) as ph1t,
                tc.tile_pool(name="pp1", bufs=1, space="PSUM") as pp1,
            ):
                wq_sb = wpool.tile([P, ND, GD], BF16)
                wk_sb = wpool.tile([P, ND, GD], BF16)
                wv_sb = wpool.tile([P, ND, GD], BF16)

                acts = {}

                def load_acts(tt):
                    xtile = ph1.tile([P, ND, P], BF16, tag="xtile",
                                     name=f"xtile{tt}")
                    nc.gpsimd.dma_start(out=xtile,
                                        in_=xt_v[:, :, tt * P:(tt + 1) * P])
                    ctile = ph1.tile([P, HG * NFREQ], BF16, tag="ctile",
                                     name=f"ctile{tt}")
                    stile = ph1.tile([P, HG * NFREQ], BF16, tag="stile",
                                     name=f"stile{tt}")
                    nc.gpsimd.dma_start(out=ctile, in_=cosb[tt * P:(tt + 1) * P, :])
                    nc.gpsimd.dma_start(out=stile, in_=sinb[tt * P:(tt + 1) * P, :])
                    acts[tt] = (xtile, ctile, stile)

                # Four DMA queues in parallel so no ordering stall: wk on
                # sync, wv on vector, wq on scalar, activations on gpsimd.
                load_acts(0)
                for d in range(ND):
                    nc.sync.dma_start(out=wk_sb[:, d, :], in_=wkt_v[:, d, :])
                    nc.sync.dma_start(out=wv_sb[:, d, :], in_=wvt_v[:, d, :])
                    nc.scalar.dma_start(out=wq_sb[:, d, :], in_=wqt_v[:, d, :])
                # Wp not needed until the first projection, far into phase 2.
                nc.sync.dma_start(out=wp_sb, in_=wpt_v)

                def qk_post(which, ps, tt):
                    """PSUM -> bf16 SBUF, raw-value RMS stats, in-place rotary
                    on the nonzero-frequency half of each head's channels.
                    Returns the finished [128, 512] bf16 tile."""
                    _, ctile, stile = acts[tt]
                    base = ph1t.tile([P, GD], BF16, tag=which + "base",
                                     bufs=4 if which == "q" else 2,
                                     name=f"{which}base{tt}")
                    if has_qkv_bias:
                        nc.vector.tensor_add(base, ps, bias_b[:, 1 if which == "k" else 0, :])
                    else:
                        nc.scalar.copy(out=base, in_=ps)
                    # stats on the raw values: rotation preserves row norms
                    sq = ph1t.tile([P, GD], BF16, tag="sq")
                    nc.vector.tensor_mul(sq, base, base)
                    ms = ph1t.tile([P, HG], F32, tag=which + "ms")
                    nc.vector.reduce_sum(out=ms,
                                         in_=sq.rearrange("p (h c) -> p h c", h=HG),
                                         axis=mybir.AxisListType.X)
                    rstd = ph1t.tile([P, HG], F32, tag=which + "rstd")
                    nc.scalar.activation(out=rstd, in_=ms, func=AF.Sqrt,
                                         scale=1.0 / HEAD_DIM, bias=eps_t[:, 0:1])
                    if which == "q":
                        nc.vector.reciprocal(out=rstd, in_=rstd)
                    else:
                        nc.vector.reciprocal(out=rk_sb[:, tt, :], in_=rstd)
                        nc.vector.tensor_scalar_mul(
                            rk_sb[:, tt, :], rk_sb[:, tt, :], SCALE)
                    # rotary on channels [0:32] & [64:96] of each head; the
                    # zero-frequency half is pass-through (already in base)
                    br = base.rearrange("p (h c) -> p h c", h=HG)
                    x1f = br[:, :, 0:NFREQ]
                    x2f = br[:, :, 2 * NFREQ:3 * NFREQ]
                    u1 = ph1t.tile([P, HG, NFREQ], BF16, tag="u1")
                    w2 = ph1t.tile([P, HG, NFREQ], BF16, tag="w2")
                    u2 = ph1t.tile([P, HG, NFREQ], BF16, tag="u2")
                    w1 = ph1t.tile([P, HG, NFREQ], BF16, tag="w1")
                    cr = ctile.rearrange("p (h c) -> p h c", h=HG)
                    sr = stile.rearrange("p (h c) -> p h c", h=HG)
                    nc.vector.tensor_mul(u1, x1f, cr)
                    nc.vector.tensor_mul(w2, x2f, sr)
                    nc.vector.tensor_mul(u2, x2f, cr)
                    nc.vector.tensor_mul(w1, x1f, sr)
                    nc.vector.tensor_add(x1f, u1, w2)
                    nc.vector.tensor_sub(x2f, u2, w1)
                    if which == "q":
                        for h in range(HG):
                            nc.vector.tensor_scalar_mul(
                                br[:, h, :], br[:, h, :], rstd[:, h:h + 1])
                    return base

                def qk_transpose(which, base, tt):
                    dst = qt_sb if which == "q" else kt_sb
                    for h in range(HG):
                        ptr = pp1.tile([P, P], BF16, tag="ptr", bufs=2)
                        nc.tensor.transpose(ptr, base[:, h * P:(h + 1) * P],
                                            identity)
                        if h % 2 == 0:
                            nc.scalar.copy(out=dst[:, tt, h, :], in_=ptr)
                        else:
                            nc.vector.tensor_copy(out=dst[:, tt, h, :], in_=ptr)

                # Software pipeline: per iteration i the PE runs K/V(i) then
                # Q(i-1), then transposes k(i) (whose post chain overlapped
                # the Q matmuls) and q(i-2) (chain finished an iteration ago).
                qbases = {}
                for i in range(NT + 2):
                    if 2 <= i:
                        qk_transpose("q", qbases.pop(i - 2), i - 2)
                    if i < NT:
                        if i + 1 < NT:
                            load_acts(i + 1)
                        xtile = acts[i][0]
                        ps_k = pp1.tile([P, GD], F32, tag="psk", bufs=2)
                        ps_v = pp1.tile([P, GD], F32, tag="psv", bufs=2)
                        for d in range(ND):
                            st, sp = d == 0, d == ND - 1
                            nc.tensor.matmul(ps_k, xtile[:, d, :], wk_sb[:, d, :],
                                             start=st, stop=sp)
                            nc.tensor.matmul(ps_v, xtile[:, d, :], wv_sb[:, d, :],
                                             start=st, stop=sp)
                        if has_qkv_bias:
                            nc.vector.tensor_add(v_sb[:, i, :], ps_v, bias_b[:, 2, :])
                        else:
                            nc.scalar.copy(out=v_sb[:, i, :], in_=ps_v)
                        kbase = qk_post("k", ps_k, i)
                    if 1 <= i <= NT:
                        tt = i - 1
                        xtile = acts[tt][0]
                        ps_q = pp1.tile([P, GD], F32, tag="psq", bufs=2)
                        for d in range(ND):
                            nc.tensor.matmul(ps_q, xtile[:, d, :], wq_sb[:, d, :],
                                             start=d == 0, stop=d == ND - 1)
                        qbases[tt] = qk_post("q", ps_q, tt)
                    if i < NT:
                        qk_transpose("k", kbase, i)

            # ---------------- Phase 2: attention + projection -----------------
            with (
                tc.tile_pool(name="ph2", bufs=2) as ph2,
                tc.tile_pool(name="pp2", bufs=1, space="PSUM") as pp2,
            ):
                def proj_groups(ic, y_sb):
                    """16 closures, each emitting one 512-col output block:
                    4 accumulating matmuls + a DVE evacuation + the DMA out.
                    Paced into the next chunk's attention loop so po banks
                    never stall the PE and the output DMA spreads out."""
                    def mk(it, dc):
                        def emit():
                            ps_o = pp2.tile([P, 512], F32, tag="po", bufs=2)
                            for co in range(HG):
                                nc.tensor.matmul(
                                    ps_o, y_sb[:, co, it * P:(it + 1) * P],
                                    wp_sb[:, co, dc * 512:(dc + 1) * 512],
                                    start=co == 0, stop=co == HG - 1)
                            o_sb = ph2.tile([P, 512], BF16, tag="o", bufs=3)
                            nc.vector.tensor_copy(out=o_sb, in_=ps_o)
                            nc.sync.dma_start(
                                out=out[(4 * ic + it) * P:(4 * ic + it + 1) * P,
                                        dc * 512:(dc + 1) * 512],
                                in_=o_sb)
                        return emit
                    return [mk(it, dc) for it in range(4) for dc in range(4)]

                pending = []
                prev_y = None
                for ic in range(NI):
                    if prev_y is not None:
                        pending = proj_groups(ic - 1, prev_y)
                    y_sb = ph2.tile([P, HG, 512], BF16, tag="y")
                    nj = 4 * (ic + 1)
                    # Projection blocks read all 4 heads of the previous
                    # chunk's y; its last pair finishes ~3us into this chunk,
                    # so hold pacing for the first 3 steps (head-of-line
                    # blocking on the in-order PE queue otherwise).
                    step_idx = 0
                    delay = 5
                    # Heads in interleaved pairs with scores+exp prefetched one
                    # j-step ahead.  Diagonal blocks (jt >= 4*ic) only compute
                    # the surviving i >= jt columns; the single partial
                    # [128,128] block gets a triangular mask.  The previous
                    # chunk's projection blocks are paced into the j-loop.
                    for hp in range(HG // 2):
                        hs = (2 * hp, 2 * hp + 1)
                        ps_ys = {h: pp2.tile([P, 512], F32, tag="py", bufs=2,
                                             name=f"ps_y{h}") for h in hs}
                        ps_ls = {h: pp2.tile([P, 512], F32, tag="pl", bufs=2,
                                             name=f"ps_l{h}") for h in hs}

                        def sc_exp(jt, h):
                            r = jt - 4 * ic
                            off = max(0, r) * P
                            ps_s = pp2.tile([P, 512], F32, tag="ps", bufs=2)
                            nc.tensor.matmul(
                                ps_s[:, off:], kt_sb[:, jt, h, :],
                                qt_sb[:, 4 * ic + max(0, r):4 * (ic + 1), h, :],
                                start=True, stop=True)
                            p_sb = ph2.tile([P, 512], BF16, tag="p", bufs=6)
                            nc.scalar.activation(
                                out=p_sb[:, off:], in_=ps_s[:, off:], func=AF.Exp,
                                scale=rk_sb[:, jt, h:h + 1])
                            if r >= 0:
                                nc.gpsimd.affine_select(
                                    out=p_sb[:, off:off + P],
                                    in_=p_sb[:, off:off + P],
                                    pattern=[[1, P]], channel_multiplier=-1,
                                    base=0,
                                    compare_op=mybir.AluOpType.is_ge, fill=0.0)
                            return p_sb, off

                        ps = {h: sc_exp(0, h) for h in hs}
                        for jt in range(nj):
                            nxt = {}
                            if jt + 1 < nj:
                                nxt = {h: sc_exp(jt + 1, h) for h in hs}
                            for h in hs:
                                st, sp = jt == 0, jt == nj - 1
                                p_sb, off = ps[h]
                                nc.tensor.matmul(ps_ls[h][:, off:], ones_t,
                                                 p_sb[:, off:],
                                                 start=st, stop=sp)
                                nc.tensor.matmul(
                                    ps_ys[h][:, off:],
                                    v_sb[:, jt, h * P:(h + 1) * P],
                                    p_sb[:, off:], start=st, stop=sp)
                            ps = nxt
                            # pace the previous chunk's projection through
                            step_idx += 1
                            if step_idx > delay and pending:
                                rem_steps = 2 * nj - step_idx + 1
                                n_emit = -(-len(pending) // rem_steps)
                                for _ in range(n_emit):
                                    pending.pop(0)()
                        # Fast PSUM evacuation (ACT frees l banks, DVE frees
                        # y banks), then the reciprocal+normalize chain runs
                        # behind on DVE — nothing waits on it until the
                        # paced projection reads y_sb several steps later.
                        l_sbs, y_raws = {}, {}
                        for h in hs:
                            l_sbs[h] = ph2.tile([P, 512], F32, tag="l_sb",
                                                name=f"l_sb{h}")
                            nc.scalar.copy(out=l_sbs[h], in_=ps_ls[h])
                            y_raws[h] = ph2.tile([P, 512], BF16, tag="y_raw",
                                                 name=f"y_raw{h}")
                            nc.vector.tensor_copy(out=y_raws[h], in_=ps_ys[h])
                        with nc.allow_low_precision(reason="2e-2 tolerance"):
                            for h in hs:
                                linv = ph2.tile([P, 512], F32, tag="linv")
                                nc.vector.reciprocal(out=linv, in_=l_sbs[h])
                                nc.vector.tensor_mul(y_sb[:, h, :], y_raws[h],
                                                     linv)
                    prev_y = y_sb
                for emit in pending:
                    emit()
                for emit in proj_groups(NI - 1, prev_y):
                    emit()
    _split_excess_waits(nc)
    return nc


_NC_CACHE = {}
_RUN_KWARGS = {}      # test harness hook: e.g. {"trace": True}
_LAST_RESULT = None   # BassKernelResults of the most recent run


def _rotary_tables():
    freq = (1.0 / 1024.0) ** np.linspace(0.0, 1.0, NFREQ, dtype=np.float32)
    theta = np.arange(T, dtype=np.float32)[:, None] * freq[None, :]      # [T, 32]
    cos = np.cos(theta).astype(np.float32)
    sin = np.sin(theta).astype(np.float32)
    cosb = np.tile(cos, (1, HG)).astype(ml_dtypes.bfloat16)              # [T, 128]
    sinb = np.tile(sin, (1, HG)).astype(ml_dtypes.bfloat16)
    return np.ascontiguousarray(cosb), np.ascontiguousarray(sinb)


def kernel(x, Wq, bq, Wk, bk, Wv, bv, Wp, bp):
    x = np.asarray(x, np.float32)
    Wq, Wk, Wv, Wp = (np.asarray(a, np.float32) for a in (Wq, Wk, Wv, Wp))
    bq, bk, bv, bp = (np.asarray(a, np.float32) for a in (bq, bk, bv, bp))

    has_bias = bool(np.any(bq) or np.any(bk) or np.any(bv))
    if has_bias not in _NC_CACHE:
        _NC_CACHE[has_bias] = _build_nc(has_bias)
    nc = _NC_CACHE[has_bias]

    def b16(a):
        return np.ascontiguousarray(a.astype(ml_dtypes.bfloat16))

    cosb, sinb = _rotary_tables()
    in_maps = []
    for c in range(NCORES):
        b, g = divmod(c, NCORES // B)
        sl = slice(g * GD, (g + 1) * GD)
        m = {
            "xt": b16(x[b].T),
            "wqt": b16(Wq[sl, :].T),
            "wkt": b16(Wk[sl, :].T),
            "wvt": b16(Wv[sl, :].T),
            "wpt": b16(Wp[:, sl].T),
            "cosb": cosb,
            "sinb": sinb,
        }
        if has_bias:
            m["bq"] = np.ascontiguousarray(bq[sl])
            m["bk"] = np.ascontiguousarray(bk[sl])
            m["bv"] = np.ascontiguousarray(bv[sl])
        in_maps.append(m)

    res = run_bass_kernel_spmd(nc, in_maps, list(range(NCORES)), **_RUN_KWARGS)
    global _LAST_RESULT
    _LAST_RESULT = res
    out = np.zeros((B, T, DIM), np.float32)
    for c in range(NCORES):
        out[c // (NCORES // B)] += res.results[c]["out"].astype(np.float32)
    out += bp[None, None, :]
    return out


# revision 16
# speedup vs baseline: 1.2449x; 1.0124x over previous
"""Trainium2 Bass kernel for causal self-attention with QK RMS-norm + rotary.

Full (unsharded) inputs in, full output out.  Internally sharded over 8
NeuronCores: data parallel on batch (2) x tensor parallel on head groups
(16 heads -> 4 groups of 4).  Each core computes q/k/v for its 4 heads on
its batch, causal flash-style attention, and a partial output projection
(its 512-column slice of Wp's input dim); the host sums the 4 partials per
batch ("all-reduce after proj" done host-side) and adds the output bias.

All matmul operands are bf16 (fp32 PSUM accumulation): same PE rate as
fp32r but with fast weight loads, half the DMA/SBUF traffic, and 2x DVE
throughput on SBUF-resident elementwise work.  Numerics validated against
the fp32 reference at ~3e-3 max-rel-err (gate 2e-2).

Per-core pipeline (single Bass program, SPMD over 8 cores):
  Phase 1, per 128-row t-tile: QKV projections with x^T tiles stationary
    (q/k/v share each weight load); ScalarE evacuates PSUM to bf16 SBUF;
    RMS stats are taken on the *raw* q/k (rotation preserves row norms) so
    they overlap the rotary; rotary touches only the 64 nonzero-frequency
    channels per head (the other half is pass-through, already in place
    from the copy); q scaled by 1/rms in place; k's norm is folded into
    the softmax exp scale (0.12/rms(k_j) per scores^T partition); q^T/k^T
    built with bf16 PE transposes and kept in SBUF (no DRAM spill).
  Phase 2, per 512-column query chunk, per head: scores^T = k^T-tile @ q^T
    so the softmax denominator comes from a ones-stationary matmul and
    attn@v needs no transpose of p; exp on ScalarE (bf16 out); causal
    diagonal blocks run at reduced moving size (only j<=i columns) with a
    GpSimd triangular affine_select on the single partial [128,128] block;
    y^T and l accumulate in PSUM over j-tiles; 1/l via the fast DVE
    reciprocal; output projection contracts the 4 head-slices of y^T
    against Wp^T, interleaved with the next chunk's attention; partial
    outputs written bf16 and reduced host-side.
"""

import os
import sys

import numpy as np

try:
    import concourse.bass as bass
except ImportError:  # fall back to the repo checkout baked into the image
    for _p in ("/opt/trn_rl_repo", "/root/.axon_site/_ro/trn_rl_repo"):
        if os.path.isdir(_p) and _p not in sys.path:
            sys.path.append(_p)
    import concourse.bass as bass

import ml_dtypes

import concourse.mybir as mybir
import concourse.tile as tile
from concourse.bass_utils import run_bass_kernel_spmd
from concourse.masks import make_identity
from concourse.vector_clock import ScopedClock

F32 = mybir.dt.float32
BF16 = mybir.dt.bfloat16
AF = mybir.ActivationFunctionType

DIM = 2048
HEAD_DIM = 128
NUM_HEADS = 16
B, T = 2, 2048
EPS = 1.1920929e-07
SCALE = 0.12

NCORES = 8
HG = 4                    # heads per core
GD = HG * HEAD_DIM        # 512: per-core q/k/v width and Wp input slice
NT = T // 128             # 16 t-tiles
ND = DIM // 128           # 16 contraction tiles
NI = T // 512             # 4 query chunks
P = 128
NFREQ = HEAD_DIM // 4     # 32 nonzero-frequency channels per head


class _TC(tile.TileContext):
    """TileContext whose final drain splits its semaphore waits across
    single-wait NOPs -- the walrus build in this image rejects CTRL
    instructions carrying 3+ sync waits ("Too many sync wait commands")."""

    def _drain_and_barrier(self, tick_clock, wait_clock):
        probe = self.nc.sync.nop(nofuse=True)
        wait_clock.add_sem_waits(probe.ins, ScopedClock({None: tick_clock.global_clock}))
        si = probe.ins.sync_info
        waits = list(si.on_wait) if si and si.on_wait else []
        if si is not None and si.on_wait:
            del si.on_wait[1:]
        for w in waits[1:]:
            nop = self.nc.sync.nop(nofuse=True)
            nsi = nop.ins.sync_info
            if nsi is None:
                nop.ins.sync_info = mybir.SyncInfo(on_wait=[w], on_update=[])
            else:
                nsi.on_wait.append(w)
        self.nc.sync.drain()
        self.nc.all_engine_barrier()
        assert self.sems is not None
        popped = self.nc._tile_sem_poison_stack.pop()
        assert popped is self._sem_poison
        self.nc.clear_and_free_semaphores(list(self.sems.allocated().values()))
        self.nc.all_engine_barrier()


_MAX_WAITS = 1


def _split_excess_waits(nc, maxw=_MAX_WAITS):
    """The walrus build in this image rejects instructions with >1 sync
    waits; spill extra waits onto NoOps inserted just before the offender
    on the same engine (all waits are preconditions, so order is free)."""
    n = 0
    for f in nc.m.functions:
        for bb in f.blocks:
            out = []
            for inst in bb.instructions:
                si = inst.sync_info
                waits = list(si.on_wait) if si and si.on_wait else []
                if len(waits) > maxw:
                    extra = waits[:-maxw]
                    del si.on_wait[: len(extra)]
                    for i in range(0, len(extra), maxw):
                        n += 1
                        nop = mybir.InstNoOp(name=f"I-wsplit-{n}-{inst.name}",
                                             ins=[], outs=[])
                        nop.engine = inst.engine
                        nop.sync_info = mybir.SyncInfo(
                            on_wait=extra[i:i + maxw], on_update=[])
                        out.append(nop)
                out.append(inst)
            bb.instructions[:] = out


def _build_nc(has_qkv_bias: bool):
    nc = bass.Bass("TRN2", target_bir_lowering=False, debug=False, num_devices=NCORES)

    xt = nc.dram_tensor("xt", [DIM, T], BF16, kind="ExternalInput")
    wqt = nc.dram_tensor("wqt", [DIM, GD], BF16, kind="ExternalInput")
    wkt = nc.dram_tensor("wkt", [DIM, GD], BF16, kind="ExternalInput")
    wvt = nc.dram_tensor("wvt", [DIM, GD], BF16, kind="ExternalInput")
    wpt = nc.dram_tensor("wpt", [GD, DIM], BF16, kind="ExternalInput")
    cosb = nc.dram_tensor("cosb", [T, HG * NFREQ], BF16, kind="ExternalInput")
    sinb = nc.dram_tensor("sinb", [T, HG * NFREQ], BF16, kind="ExternalInput")
    if has_qkv_bias:
        bq = nc.dram_tensor("bq", [GD], F32, kind="ExternalInput")
        bk = nc.dram_tensor("bk", [GD], F32, kind="ExternalInput")
        bv = nc.dram_tensor("bv", [GD], F32, kind="ExternalInput")
    out = nc.dram_tensor("out", [T, DIM], BF16, kind="ExternalOutput")

    xt_v = xt.rearrange("(do p) t -> p do t", p=P)      # [128, 16, 2048]
    wqt_v = wqt.rearrange("(do p) o -> p do o", p=P)    # [128, 16, 512]
    wkt_v = wkt.rearrange("(do p) o -> p do o", p=P)
    wvt_v = wvt.rearrange("(do p) o -> p do o", p=P)
    wpt_v = wpt.rearrange("(co p) o -> p co o", p=P)    # [128, 4, 2048]

    with _TC(nc) as tc:
        with (
            tc.tile_pool(name="const", bufs=1) as constp,
            tc.tile_pool(name="persist", bufs=1) as persist,
        ):
            identity = constp.tile([P, P], BF16)
            make_identity(nc, identity)
            ones_t = constp.tile([P, P], BF16)
            nc.vector.memset(ones_t, 1.0)
            eps_t = constp.tile([P, 1], F32)
            nc.vector.memset(eps_t, EPS)
            if has_qkv_bias:
                bias_b = constp.tile([P, 3, GD], F32)
                for bi, bten in enumerate((bq, bk, bv)):
                    bcast = bass.AP(tensor=bten.tensor, offset=bten.offset,
                                    ap=[[0, P]] + list(bten.ap))
                    nc.sync.dma_start(out=bias_b[:, bi, :], in_=bcast)

            v_sb = persist.tile([P, NT, GD], BF16)       # v, natural [t, head*128]
            rk_sb = persist.tile([P, NT, HG], F32)       # 0.12/rms(k) per (t, head)
            qt_sb = persist.tile([P, NT, HG, P], BF16)   # q^T [c, tt, h, t]
            kt_sb = persist.tile([P, NT, HG, P], BF16)
            wp_sb = persist.tile([P, HG, DIM], BF16)

            # ---------------- Phase 1: QKV + rotary + norms + transposes ------
            # K/V for t-tile i and Q for t-tile i-1 per iteration: Q trails
            # one tile so the PE starts on K/V as soon as the first weight
            # chunks land instead of waiting for all three weight matrices.
            with (
                tc.tile_pool(name="wqkv", bufs=1) as wpool,
                tc.tile_pool(name="ph1", bufs=3) as ph1,
                tc.tile_pool(name="ph1t", bufs=2) as ph1t,
                tc.tile_pool(name="pp1", bufs=1, space="PSUM") as pp1,
            ):
                wq_sb = wpool.tile([P, ND, GD], BF16)
                wk_sb = wpool.tile([P, ND, GD], BF16)
                wv_sb = wpool.tile([P, ND, GD], BF16)

                acts = {}

                def load_acts(tt):
                    xtile = ph1.tile([P, ND, P], BF16, tag="xtile",
                                     name=f"xtile{tt}")
                    if tt == 0:
                        # split so the first matmul gates on 128KB, not 512KB
                        nc.gpsimd.dma_start(out=xtile[:, 0:4, :],
                                            in_=xt_v[:, 0:4, 0:P])
                        nc.gpsimd.dma_start(out=xtile[:, 4:, :],
                                            in_=xt_v[:, 4:, 0:P])
                    else:
                        nc.gpsimd.dma_start(out=xtile,
                                            in_=xt_v[:, :, tt * P:(tt + 1) * P])
                    ctile = ph1.tile([P, HG * NFREQ], BF16, tag="ctile",
                                     name=f"ctile{tt}")
                    stile = ph1.tile([P, HG * NFREQ], BF16, tag="stile",
                                     name=f"stile{tt}")
                    nc.gpsimd.dma_start(out=ctile, in_=cosb[tt * P:(tt + 1) * P, :])
                    nc.gpsimd.dma_start(out=stile, in_=sinb[tt * P:(tt + 1) * P, :])
                    acts[tt] = (xtile, ctile, stile)

                # Four DMA queues in parallel so no ordering stall: wk on
                # sync, wv on vector, wq on scalar, activations on gpsimd.
                load_acts(0)
                for d in range(ND):
                    nc.sync.dma_start(out=wk_sb[:, d, :], in_=wkt_v[:, d, :])
                    nc.sync.dma_start(out=wv_sb[:, d, :], in_=wvt_v[:, d, :])
                    nc.scalar.dma_start(out=wq_sb[:, d, :], in_=wqt_v[:, d, :])
                # Wp not needed until the first projection, far into phase 2.
                nc.sync.dma_start(out=wp_sb, in_=wpt_v)

                def qk_post(which, ps, tt):
                    """PSUM -> bf16 SBUF, raw-value RMS stats, in-place rotary
                    on the nonzero-frequency half of each head's channels.
                    Returns the finished [128, 512] bf16 tile."""
                    _, ctile, stile = acts[tt]
                    base = ph1t.tile([P, GD], BF16, tag=which + "base",
                                     bufs=4 if which == "q" else 2,
                                     name=f"{which}base{tt}")
                    if has_qkv_bias:
                        nc.vector.tensor_add(base, ps, bias_b[:, 1 if which == "k" else 0, :])
                    else:
                        nc.scalar.copy(out=base, in_=ps)
                    # stats on the raw values: rotation preserves row norms
                    sq = ph1t.tile([P, GD], BF16, tag="sq")
                    nc.vector.tensor_mul(sq, base, base)
                    ms = ph1t.tile([P, HG], F32, tag=which + "ms")
                    nc.vector.reduce_sum(out=ms,
                                         in_=sq.rearrange("p (h c) -> p h c", h=HG),
                                         axis=mybir.AxisListType.X)
                    rstd = ph1t.tile([P, HG], F32, tag=which + "rstd")
                    nc.scalar.activation(out=rstd, in_=ms, func=AF.Sqrt,
                                         scale=1.0 / HEAD_DIM, bias=eps_t[:, 0:1])
                    if which == "q":
                        nc.vector.reciprocal(out=rstd, in_=rstd)
                    else:
                        nc.vector.reciprocal(out=rk_sb[:, tt, :], in_=rstd)
                        nc.vector.tensor_scalar_mul(
                            rk_sb[:, tt, :], rk_sb[:, tt, :], SCALE)
                    # rotary on channels [0:32] & [64:96] of each head; the
                    # zero-frequency half is pass-through (already in base)
                    br = base.rearrange("p (h c) -> p h c", h=HG)
                    x1f = br[:, :, 0:NFREQ]
                    x2f = br[:, :, 2 * NFREQ:3 * NFREQ]
                    u1 = ph1t.tile([P, HG, NFREQ], BF16, tag="u1")
                    w2 = ph1t.tile([P, HG, NFREQ], BF16, tag="w2")
                    u2 = ph1t.tile([P, HG, NFREQ], BF16, tag="u2")
                    w1 = ph1t.tile([P, HG, NFREQ], BF16, tag="w1")
                    cr = ctile.rearrange("p (h c) -> p h c", h=HG)
                    sr = stile.rearrange("p (h c) -> p h c", h=HG)
                    nc.vector.tensor_mul(u1, x1f, cr)
                    nc.vector.tensor_mul(w2, x2f, sr)
                    nc.vector.tensor_mul(u2, x2f, cr)
                    nc.vector.tensor_mul(w1, x1f, sr)
                    nc.vector.tensor_add(x1f, u1, w2)
                    nc.vector.tensor_sub(x2f, u2, w1)
                    if which == "q":
                        for h in range(HG):
                            nc.vector.tensor_scalar_mul(
                                br[:, h, :], br[:, h, :], rstd[:, h:h + 1])
                    return base

                def qk_transpose(which, base, tt):
                    dst = qt_sb if which == "q" else kt_sb
                    for h in range(HG):
                        ptr = pp1.tile([P, P], BF16, tag="ptr", bufs=2)
                        nc.tensor.transpose(ptr, base[:, h * P:(h + 1) * P],
                                            identity)
                        if h % 2 == 0:
                            nc.scalar.copy(out=dst[:, tt, h, :], in_=ptr)
                        else:
                            nc.vector.tensor_copy(out=dst[:, tt, h, :], in_=ptr)

                # Software pipeline: per iteration i the PE runs K/V(i) then
                # Q(i-1), then transposes k(i) (whose post chain overlapped
                # the Q matmuls) and q(i-2) (chain finished an iteration ago).
                qbases = {}
                for i in range(NT + 2):
                    if 2 <= i:
                        qk_transpose("q", qbases.pop(i - 2), i - 2)
                    if i < NT:
                        if i + 1 < NT:
                            load_acts(i + 1)
                        xtile = acts[i][0]
                        ps_k = pp1.tile([P, GD], F32, tag="psk", bufs=2)
                        ps_v = pp1.tile([P, GD], F32, tag="psv", bufs=2)
                        for d in range(ND):
                            st, sp = d == 0, d == ND - 1
                            nc.tensor.matmul(ps_k, xtile[:, d, :], wk_sb[:, d, :],
                                             start=st, stop=sp)
                            nc.tensor.matmul(ps_v, xtile[:, d, :], wv_sb[:, d, :],
                                             start=st, stop=sp)
                        if has_qkv_bias:
                            nc.vector.tensor_add(v_sb[:, i, :], ps_v, bias_b[:, 2, :])
                        else:
                            nc.scalar.copy(out=v_sb[:, i, :], in_=ps_v)
                        kbase = qk_post("k", ps_k, i)
                    if 1 <= i <= NT:
                        tt = i - 1
                        xtile = acts[tt][0]
                        ps_q = pp1.tile([P, GD], F32, tag="psq", bufs=2)
                        for d in range(ND):
                            nc.tensor.matmul(ps_q, xtile[:, d, :], wq_sb[:, d, :],
                                             start=d == 0, stop=d == ND - 1)
                        qbases[tt] = qk_post("q", ps_q, tt)
                    if i < NT:
                        qk_transpose("k", kbase, i)

            # ---------------- Phase 2: attention + projection -----------------
            with (
                tc.tile_pool(name="ph2", bufs=2) as ph2,
                tc.tile_pool(name="pp2", bufs=1, space="PSUM") as pp2,
            ):
                def proj_groups(ic, y_sb):
                    """16 closures, each emitting one 512-col output block:
                    4 accumulating matmuls + a DVE evacuation + the DMA out.
                    Paced into the next chunk's attention loop so po banks
                    never stall the PE and the output DMA spreads out."""
                    def mk(it, dc):
                        def emit():
                            ps_o = pp2.tile([P, 512], F32, tag="po", bufs=2)
                            for co in range(HG):
                                nc.tensor.matmul(
                                    ps_o, y_sb[:, co, it * P:(it + 1) * P],
                                    wp_sb[:, co, dc * 512:(dc + 1) * 512],
                                    start=co == 0, stop=co == HG - 1)
                            o_sb = ph2.tile([P, 512], BF16, tag="o", bufs=3)
                            nc.vector.tensor_copy(out=o_sb, in_=ps_o)
                            nc.sync.dma_start(
                                out=out[(4 * ic + it) * P:(4 * ic + it + 1) * P,
                                        dc * 512:(dc + 1) * 512],
                                in_=o_sb)
                        return emit
                    return [mk(it, dc) for it in range(4) for dc in range(4)]

                pending = []
                prev_y = None
                for ic in range(NI):
                    if prev_y is not None:
                        pending = proj_groups(ic - 1, prev_y)
                    y_sb = ph2.tile([P, HG, 512], BF16, tag="y")
                    nj = 4 * (ic + 1)
                    # Projection blocks read all 4 heads of the previous
                    # chunk's y; its last pair finishes ~3us into this chunk,
                    # so hold pacing for the first 3 steps (head-of-line
                    # blocking on the in-order PE queue otherwise).
                    step_idx = 0
                    delay = 7
                    # Heads in interleaved pairs with scores+exp prefetched one
                    # j-step ahead.  Diagonal blocks (jt >= 4*ic) only compute
                    # the surviving i >= jt columns; the single partial
                    # [128,128] block gets a triangular mask.  The previous
                    # chunk's projection blocks are paced into the j-loop.
                    for hp in range(HG // 2):
                        hs = (2 * hp, 2 * hp + 1)
                        ps_ys = {h: pp2.tile([P, 512], F32, tag="py", bufs=2,
                                             name=f"ps_y{h}") for h in hs}
                        ps_ls = {h: pp2.tile([P, 512], F32, tag="pl", bufs=2,
                                             name=f"ps_l{h}") for h in hs}

                        def sc_exp(jt, h):
                            r = jt - 4 * ic
                            off = max(0, r) * P
                            ps_s = pp2.tile([P, 512], F32, tag="ps", bufs=2)
                            nc.tensor.matmul(
                                ps_s[:, off:], kt_sb[:, jt, h, :],
                                qt_sb[:, 4 * ic + max(0, r):4 * (ic + 1), h, :],
                                start=True, stop=True)
                            p_sb = ph2.tile([P, 512], BF16, tag="p", bufs=6)
                            nc.scalar.activation(
                                out=p_sb[:, off:], in_=ps_s[:, off:], func=AF.Exp,
                                scale=rk_sb[:, jt, h:h + 1])
                            if r >= 0:
                                nc.gpsimd.affine_select(
                                    out=p_sb[:, off:off + P],
                                    in_=p_sb[:, off:off + P],
                                    pattern=[[1, P]], channel_multiplier=-1,
                                    base=0,
                                    compare_op=mybir.AluOpType.is_ge, fill=0.0)
                            return p_sb, off

                        ps = {h: sc_exp(0, h) for h in hs}
                        for jt in range(nj):
                            nxt = {}
                            if jt + 1 < nj:
                                nxt = {h: sc_exp(jt + 1, h) for h in hs}
                            for h in hs:
                                st, sp = jt == 0, jt == nj - 1
                                p_sb, off = ps[h]
                                nc.tensor.matmul(ps_ls[h][:, off:], ones_t,
                                                 p_sb[:, off:],
                                                 start=st, stop=sp)
                                nc.tensor.matmul(
                                    ps_ys[h][:, off:],
                                    v_sb[:, jt, h * P:(h + 1) * P],
                                    p_sb[:, off:], start=st, stop=sp)
                            ps = nxt
                            # pace the previous chunk's projection through
                            step_idx += 1
                            if step_idx > delay and pending:
                                rem_steps = 2 * nj - step_idx + 1
                                n_emit = -(-len(pending) // rem_steps)
                                for _ in range(n_emit):
                                    pending.pop(0)()
                        # Fast PSUM evacuation (ACT frees l banks, DVE frees
                        # y banks), then the reciprocal+normalize chain runs
                        # behind on DVE — nothing waits on it until the
                        # paced projection reads y_sb several steps later.
                        # The very last pair skips the y copies (no next
                        # pair needs the banks) to shorten the tail chain.
                        tail = ic == NI - 1 and hp == HG // 2 - 1
                        l_sbs, y_raws = {}, {}
                        for h in hs:
                            l_sbs[h] = ph2.tile([P, 512], F32, tag="l_sb",
                                                name=f"l_sb{h}")
                            nc.scalar.copy(out=l_sbs[h], in_=ps_ls[h])
                            if not tail:
                                y_raws[h] = ph2.tile([P, 512], BF16,
                                                     tag="y_raw",
                                                     name=f"y_raw{h}")
                                nc.vector.tensor_copy(out=y_raws[h],
                                                      in_=ps_ys[h])
                        with nc.allow_low_precision(reason="2e-2 tolerance"):
                            for h in hs:
                                linv = ph2.tile([P, 512], F32, tag="linv")
                                nc.vector.reciprocal(out=linv, in_=l_sbs[h])
                                nc.vector.tensor_mul(
                                    y_sb[:, h, :],
                                    ps_ys[h] if tail else y_raws[h], linv)
                    prev_y = y_sb
                for emit in pending:
                    emit()
                for emit in proj_groups(NI - 1, prev_y):
                    emit()
    _split_excess_waits(nc)
    return nc


_NC_CACHE = {}
_RUN_KWARGS = {}      # test harness hook: e.g. {"trace": True}
_LAST_RESULT = None   # BassKernelResults of the most recent run


def _rotary_tables():
    freq = (1.0 / 1024.0) ** np.linspace(0.0, 1.0, NFREQ, dtype=np.float32)
    theta = np.arange(T, dtype=np.float32)[:, None] * freq[None, :]      # [T, 32]
    cos = np.cos(theta).astype(np.float32)
    sin = np.sin(theta).astype(np.float32)
    cosb = np.tile(cos, (1, HG)).astype(ml_dtypes.bfloat16)              # [T, 128]
    sinb = np.tile(sin, (1, HG)).astype(ml_dtypes.bfloat16)
    return np.ascontiguousarray(cosb), np.ascontiguousarray(sinb)


def kernel(x, Wq, bq, Wk, bk, Wv, bv, Wp, bp):
    x = np.asarray(x, np.float32)
    Wq, Wk, Wv, Wp = (np.asarray(a, np.float32) for a in (Wq, Wk, Wv, Wp))
    bq, bk, bv, bp = (np.asarray(a, np.float32) for a in (bq, bk, bv, bp))

    has_bias = bool(np.any(bq) or np.any(bk) or np.any(bv))
    if has_bias not in _NC_CACHE:
        _NC_CACHE[has_bias] = _build_nc(has_bias)
    nc = _NC_CACHE[has_bias]

    def b16(a):
        return np.ascontiguousarray(a.astype(ml_dtypes.bfloat16))

    cosb, sinb = _rotary_tables()
    in_maps = []
    for c in range(NCORES):
        b, g = divmod(c, NCORES // B)
        sl = slice(g * GD, (g + 1) * GD)
        m = {
            "xt": b16(x[b].T),
            "wqt": b16(Wq[sl, :].T),
            "wkt": b16(Wk[sl, :].T),
            "wvt": b16(Wv[sl, :].T),
            "wpt": b16(Wp[:, sl].T),
            "cosb": cosb,
            "sinb": sinb,
        }
        if has_bias:
            m["bq"] = np.ascontiguousarray(bq[sl])
            m["bk"] = np.ascontiguousarray(bk[sl])
            m["bv"] = np.ascontiguousarray(bv[sl])
        in_maps.append(m)

    res = run_bass_kernel_spmd(nc, in_maps, list(range(NCORES)), **_RUN_KWARGS)
    global _LAST_RESULT
    _LAST_RESULT = res
    out = np.zeros((B, T, DIM), np.float32)
    for c in range(NCORES):
        out[c // (NCORES // B)] += res.results[c]["out"].astype(np.float32)
    out += bp[None, None, :]
    return out


# revision 17
# speedup vs baseline: 1.3437x; 1.0794x over previous
"""Trainium2 Bass kernel for causal self-attention with QK RMS-norm + rotary.

Full (unsharded) inputs in, full output out.  Internally sharded over 8
NeuronCores: data parallel on batch (2) x tensor parallel on head groups
(16 heads -> 4 groups of 4).  Each core computes q/k/v for its 4 heads on
its batch, causal flash-style attention, and a partial output projection
(its 512-column slice of Wp's input dim); the host sums the 4 partials per
batch ("all-reduce after proj" done host-side) and adds the output bias.

All matmul operands are bf16 (fp32 PSUM accumulation): same PE rate as
fp32r but with fast weight loads, half the DMA/SBUF traffic, and 2x DVE
throughput on SBUF-resident elementwise work.  Numerics validated against
the fp32 reference at ~3e-3 max-rel-err (gate 2e-2).

Per-core pipeline (single Bass program, SPMD over 8 cores):
  Phase 1, per 128-row t-tile: QKV projections with x^T tiles stationary
    (q/k/v share each weight load); ScalarE evacuates PSUM to bf16 SBUF;
    RMS stats are taken on the *raw* q/k (rotation preserves row norms) so
    they overlap the rotary; rotary touches only the 64 nonzero-frequency
    channels per head (the other half is pass-through, already in place
    from the copy); q scaled by 1/rms in place; k's norm is folded into
    the softmax exp scale (0.12/rms(k_j) per scores^T partition); q^T/k^T
    built with bf16 PE transposes and kept in SBUF (no DRAM spill).
  Phase 2, per 512-column query chunk, per head: scores^T = k^T-tile @ q^T
    so the softmax denominator comes from a ones-stationary matmul and
    attn@v needs no transpose of p; exp on ScalarE (bf16 out); causal
    diagonal blocks run at reduced moving size (only j<=i columns) with a
    GpSimd triangular affine_select on the single partial [128,128] block;
    y^T and l accumulate in PSUM over j-tiles; 1/l via the fast DVE
    reciprocal; output projection contracts the 4 head-slices of y^T
    against Wp^T, interleaved with the next chunk's attention; partial
    outputs written bf16 and reduced host-side.
"""

import os
import sys

import numpy as np

try:
    import concourse.bass as bass
except ImportError:  # fall back to the repo checkout baked into the image
    for _p in ("/opt/trn_rl_repo", "/root/.axon_site/_ro/trn_rl_repo"):
        if os.path.isdir(_p) and _p not in sys.path:
            sys.path.append(_p)
    import concourse.bass as bass

import ml_dtypes

import concourse.mybir as mybir
import concourse.tile as tile
from concourse.bass_utils import run_bass_kernel_spmd
from concourse.masks import make_identity
from concourse.vector_clock import ScopedClock

F32 = mybir.dt.float32
BF16 = mybir.dt.bfloat16
AF = mybir.ActivationFunctionType

DIM = 2048
HEAD_DIM = 128
NUM_HEADS = 16
B, T = 2, 2048
EPS = 1.1920929e-07
SCALE = 0.12

NCORES = 8
HG = 4                    # heads per core
GD = HG * HEAD_DIM        # 512: per-core q/k/v width and Wp input slice
NT = T // 128             # 16 t-tiles
ND = DIM // 128           # 16 contraction tiles
NI = T // 512             # 4 query chunks
P = 128
NFREQ = HEAD_DIM // 4     # 32 nonzero-frequency channels per head


class _TC(tile.TileContext):
    """TileContext whose final drain splits its semaphore waits across
    single-wait NOPs -- the walrus build in this image rejects CTRL
    instructions carrying 3+ sync waits ("Too many sync wait commands")."""

    def _drain_and_barrier(self, tick_clock, wait_clock):
        probe = self.nc.sync.nop(nofuse=True)
        wait_clock.add_sem_waits(probe.ins, ScopedClock({None: tick_clock.global_clock}))
        si = probe.ins.sync_info
        waits = list(si.on_wait) if si and si.on_wait else []
        if si is not None and si.on_wait:
            del si.on_wait[1:]
        for w in waits[1:]:
            nop = self.nc.sync.nop(nofuse=True)
            nsi = nop.ins.sync_info
            if nsi is None:
                nop.ins.sync_info = mybir.SyncInfo(on_wait=[w], on_update=[])
            else:
                nsi.on_wait.append(w)
        self.nc.sync.drain()
        self.nc.all_engine_barrier()
        assert self.sems is not None
        popped = self.nc._tile_sem_poison_stack.pop()
        assert popped is self._sem_poison
        self.nc.clear_and_free_semaphores(list(self.sems.allocated().values()))
        self.nc.all_engine_barrier()


_MAX_WAITS = 1


def _split_excess_waits(nc, maxw=_MAX_WAITS):
    """The walrus build in this image rejects instructions with >1 sync
    waits; spill extra waits onto NoOps inserted just before the offender
    on the same engine (all waits are preconditions, so order is free)."""
    n = 0
    for f in nc.m.functions:
        for bb in f.blocks:
            out = []
            for inst in bb.instructions:
                si = inst.sync_info
                waits = list(si.on_wait) if si and si.on_wait else []
                if len(waits) > maxw:
                    extra = waits[:-maxw]
                    del si.on_wait[: len(extra)]
                    for i in range(0, len(extra), maxw):
                        n += 1
                        nop = mybir.InstNoOp(name=f"I-wsplit-{n}-{inst.name}",
                                             ins=[], outs=[])
                        nop.engine = inst.engine
                        nop.sync_info = mybir.SyncInfo(
                            on_wait=extra[i:i + maxw], on_update=[])
                        out.append(nop)
                out.append(inst)
            bb.instructions[:] = out


def _build_nc(has_qkv_bias: bool):
    nc = bass.Bass("TRN2", target_bir_lowering=False, debug=False, num_devices=NCORES)

    xt = nc.dram_tensor("xt", [DIM, T], BF16, kind="ExternalInput")
    wqt = nc.dram_tensor("wqt", [DIM, GD], BF16, kind="ExternalInput")
    wkt = nc.dram_tensor("wkt", [DIM, GD], BF16, kind="ExternalInput")
    wvt = nc.dram_tensor("wvt", [DIM, GD], BF16, kind="ExternalInput")
    wpt = nc.dram_tensor("wpt", [GD, DIM], BF16, kind="ExternalInput")
    cosb = nc.dram_tensor("cosb", [T, HG * NFREQ], BF16, kind="ExternalInput")
    sinb = nc.dram_tensor("sinb", [T, HG * NFREQ], BF16, kind="ExternalInput")
    if has_qkv_bias:
        bq = nc.dram_tensor("bq", [GD], F32, kind="ExternalInput")
        bk = nc.dram_tensor("bk", [GD], F32, kind="ExternalInput")
        bv = nc.dram_tensor("bv", [GD], F32, kind="ExternalInput")
    out = nc.dram_tensor("out", [T, DIM], BF16, kind="ExternalOutput")

    xt_v = xt.rearrange("(do p) t -> p do t", p=P)      # [128, 16, 2048]
    wqt_v = wqt.rearrange("(do p) o -> p do o", p=P)    # [128, 16, 512]
    wkt_v = wkt.rearrange("(do p) o -> p do o", p=P)
    wvt_v = wvt.rearrange("(do p) o -> p do o", p=P)
    wpt_v = wpt.rearrange("(co p) o -> p co o", p=P)    # [128, 4, 2048]

    with _TC(nc) as tc:
        with (
            tc.tile_pool(name="const", bufs=1) as constp,
            tc.tile_pool(name="persist", bufs=1) as persist,
        ):
            identity = constp.tile([P, P], BF16)
            make_identity(nc, identity)
            ones_t = constp.tile([P, P], BF16)
            nc.vector.memset(ones_t, 1.0)
            eps_t = constp.tile([P, 1], F32)
            nc.vector.memset(eps_t, EPS)
            if has_qkv_bias:
                bias_b = constp.tile([P, 3, GD], F32)
                for bi, bten in enumerate((bq, bk, bv)):
                    bcast = bass.AP(tensor=bten.tensor, offset=bten.offset,
                                    ap=[[0, P]] + list(bten.ap))
                    nc.sync.dma_start(out=bias_b[:, bi, :], in_=bcast)

            v_sb = persist.tile([P, NT, GD], BF16)       # v, natural [t, head*128]
            rk_sb = persist.tile([P, NT, HG], F32)       # 0.12/rms(k) per (t, head)
            qt_sb = persist.tile([P, NT, HG, P], BF16)   # q^T [c, tt, h, t]
            kt_sb = persist.tile([P, NT, HG, P], BF16)
            wp_sb = persist.tile([P, HG, DIM], BF16)

            # ---------------- Phase 1: QKV + rotary + norms + transposes ------
            # K/V for t-tile i and Q for t-tile i-1 per iteration: Q trails
            # one tile so the PE starts on K/V as soon as the first weight
            # chunks land instead of waiting for all three weight matrices.
            with (
                tc.tile_pool(name="wqkv", bufs=1) as wpool,
                tc.tile_pool(name="ph1", bufs=3) as ph1,
                tc.tile_pool(name="ph1t", bufs=2) as ph1t,
                tc.tile_pool(name="pp1", bufs=1, space="PSUM") as pp1,
            ):
                wq_sb = wpool.tile([P, ND, GD], BF16)
                wk_sb = wpool.tile([P, ND, GD], BF16)
                wv_sb = wpool.tile([P, ND, GD], BF16)

                acts = {}

                def load_acts(tt):
                    xtile = ph1.tile([P, ND, P], BF16, tag="xtile",
                                     name=f"xtile{tt}")
                    if tt == 0:
                        # split so the first matmul gates on 128KB, not 512KB
                        nc.gpsimd.dma_start(out=xtile[:, 0:4, :],
                                            in_=xt_v[:, 0:4, 0:P])
                        nc.gpsimd.dma_start(out=xtile[:, 4:, :],
                                            in_=xt_v[:, 4:, 0:P])
                    else:
                        nc.gpsimd.dma_start(out=xtile,
                                            in_=xt_v[:, :, tt * P:(tt + 1) * P])
                    ctile = ph1.tile([P, HG * NFREQ], BF16, tag="ctile",
                                     name=f"ctile{tt}")
                    stile = ph1.tile([P, HG * NFREQ], BF16, tag="stile",
                                     name=f"stile{tt}")
                    nc.gpsimd.dma_start(out=ctile, in_=cosb[tt * P:(tt + 1) * P, :])
                    nc.gpsimd.dma_start(out=stile, in_=sinb[tt * P:(tt + 1) * P, :])
                    acts[tt] = (xtile, ctile, stile)

                # Four DMA queues in parallel so no ordering stall: wk on
                # sync, wv on vector, wq on scalar, activations on gpsimd.
                load_acts(0)
                for d in range(ND):
                    nc.sync.dma_start(out=wk_sb[:, d, :], in_=wkt_v[:, d, :])
                    nc.sync.dma_start(out=wv_sb[:, d, :], in_=wvt_v[:, d, :])
                    nc.scalar.dma_start(out=wq_sb[:, d, :], in_=wqt_v[:, d, :])
                # Wp not needed until the first projection, far into phase 2.
                nc.sync.dma_start(out=wp_sb, in_=wpt_v)

                def qk_post(which, ps, tt):
                    """PSUM -> bf16 SBUF, raw-value RMS stats, in-place rotary
                    on the nonzero-frequency half of each head's channels.
                    Returns the finished [128, 512] bf16 tile."""
                    _, ctile, stile = acts[tt]
                    base = ph1t.tile([P, GD], BF16, tag=which + "base",
                                     bufs=4 if which == "q" else 2,
                                     name=f"{which}base{tt}")
                    if has_qkv_bias:
                        nc.vector.tensor_add(base, ps, bias_b[:, 1 if which == "k" else 0, :])
                    else:
                        nc.scalar.copy(out=base, in_=ps)
                    # stats on the raw values: rotation preserves row norms
                    sq = ph1t.tile([P, GD], BF16, tag="sq")
                    nc.vector.tensor_mul(sq, base, base)
                    ms = ph1t.tile([P, HG], F32, tag=which + "ms")
                    nc.vector.reduce_sum(out=ms,
                                         in_=sq.rearrange("p (h c) -> p h c", h=HG),
                                         axis=mybir.AxisListType.X)
                    rstd = ph1t.tile([P, HG], F32, tag=which + "rstd")
                    nc.scalar.activation(out=rstd, in_=ms, func=AF.Sqrt,
                                         scale=1.0 / HEAD_DIM, bias=eps_t[:, 0:1])
                    if which == "q":
                        nc.vector.reciprocal(out=rstd, in_=rstd)
                    else:
                        nc.vector.reciprocal(out=rk_sb[:, tt, :], in_=rstd)
                        nc.vector.tensor_scalar_mul(
                            rk_sb[:, tt, :], rk_sb[:, tt, :], SCALE)
                    # rotary on channels [0:32] & [64:96] of each head; the
                    # zero-frequency half is pass-through (already in base)
                    br = base.rearrange("p (h c) -> p h c", h=HG)
                    x1f = br[:, :, 0:NFREQ]
                    x2f = br[:, :, 2 * NFREQ:3 * NFREQ]
                    u1 = ph1t.tile([P, HG, NFREQ], BF16, tag="u1")
                    w2 = ph1t.tile([P, HG, NFREQ], BF16, tag="w2")
                    u2 = ph1t.tile([P, HG, NFREQ], BF16, tag="u2")
                    w1 = ph1t.tile([P, HG, NFREQ], BF16, tag="w1")
                    cr = ctile.rearrange("p (h c) -> p h c", h=HG)
                    sr = stile.rearrange("p (h c) -> p h c", h=HG)
                    nc.vector.tensor_mul(u1, x1f, cr)
                    nc.vector.tensor_mul(w2, x2f, sr)
                    nc.vector.tensor_mul(u2, x2f, cr)
                    nc.vector.tensor_mul(w1, x1f, sr)
                    nc.vector.tensor_add(x1f, u1, w2)
                    nc.vector.tensor_sub(x2f, u2, w1)
                    if which == "q":
                        for h in range(HG):
                            nc.vector.tensor_scalar_mul(
                                br[:, h, :], br[:, h, :], rstd[:, h:h + 1])
                    return base

                def qk_transpose(which, base, tt):
                    dst = qt_sb if which == "q" else kt_sb
                    for h in range(HG):
                        ptr = pp1.tile([P, P], BF16, tag="ptr", bufs=2)
                        nc.tensor.transpose(ptr, base[:, h * P:(h + 1) * P],
                                            identity)
                        if h % 2 == 0:
                            nc.scalar.copy(out=dst[:, tt, h, :], in_=ptr)
                        else:
                            nc.vector.tensor_copy(out=dst[:, tt, h, :], in_=ptr)

                # Software pipeline: per iteration i the PE runs K/V(i) then
                # Q(i-1), then transposes k(i) (whose post chain overlapped
                # the Q matmuls) and q(i-2) (chain finished an iteration ago).
                qbases = {}
                for i in range(NT + 2):
                    if 2 <= i:
                        qk_transpose("q", qbases.pop(i - 2), i - 2)
                    if i < NT:
                        if i + 1 < NT:
                            load_acts(i + 1)
                        xtile = acts[i][0]
                        ps_k = pp1.tile([P, GD], F32, tag="psk", bufs=2)
                        ps_v = pp1.tile([P, GD], F32, tag="psv", bufs=2)
                        for d in range(ND):
                            st, sp = d == 0, d == ND - 1
                            nc.tensor.matmul(ps_k, xtile[:, d, :], wk_sb[:, d, :],
                                             start=st, stop=sp)
                            nc.tensor.matmul(ps_v, xtile[:, d, :], wv_sb[:, d, :],
                                             start=st, stop=sp)
                        if has_qkv_bias:
                            nc.vector.tensor_add(v_sb[:, i, :], ps_v, bias_b[:, 2, :])
                        else:
                            nc.scalar.copy(out=v_sb[:, i, :], in_=ps_v)
                        kbase = qk_post("k", ps_k, i)
                    if 1 <= i <= NT:
                        tt = i - 1
                        xtile = acts[tt][0]
                        ps_q = pp1.tile([P, GD], F32, tag="psq", bufs=2)
                        for d in range(ND):
                            nc.tensor.matmul(ps_q, xtile[:, d, :], wq_sb[:, d, :],
                                             start=d == 0, stop=d == ND - 1)
                        qbases[tt] = qk_post("q", ps_q, tt)
                    if i < NT:
                        qk_transpose("k", kbase, i)

            # ---------------- Phase 2: attention + projection -----------------
            with (
                tc.tile_pool(name="ph2", bufs=2) as ph2,
                tc.tile_pool(name="pp2", bufs=1, space="PSUM") as pp2,
            ):
                def proj_groups(ic, y_sb):
                    """16 closures, each emitting one 512-col output block:
                    4 accumulating matmuls + a DVE evacuation + the DMA out.
                    Paced into the next chunk's attention loop so po banks
                    never stall the PE and the output DMA spreads out."""
                    def mk(it, dc):
                        def emit():
                            ps_o = pp2.tile([P, 512], F32, tag="po", bufs=2)
                            for co in range(HG):
                                nc.tensor.matmul(
                                    ps_o, y_sb[:, co, it * P:(it + 1) * P],
                                    wp_sb[:, co, dc * 512:(dc + 1) * 512],
                                    start=co == 0, stop=co == HG - 1)
                            o_sb = ph2.tile([P, 512], BF16, tag="o", bufs=3)
                            nc.vector.tensor_copy(out=o_sb, in_=ps_o)
                            nc.sync.dma_start(
                                out=out[(4 * ic + it) * P:(4 * ic + it + 1) * P,
                                        dc * 512:(dc + 1) * 512],
                                in_=o_sb)
                        return emit
                    return [mk(it, dc) for it in range(4) for dc in range(4)]

                pending = []
                prev_y = None
                for ic in range(NI):
                    if prev_y is not None:
                        pending = proj_groups(ic - 1, prev_y)
                    y_sb = ph2.tile([P, HG, 512], BF16, tag="y")
                    nj = 4 * (ic + 1)
                    # Projection blocks read all 4 heads of the previous
                    # chunk's y; its last pair finishes ~3us into this chunk,
                    # so hold pacing for the first 3 steps (head-of-line
                    # blocking on the in-order PE queue otherwise).
                    step_idx = 0
                    delay = 7
                    # Heads in interleaved pairs with scores+exp prefetched one
                    # j-step ahead.  Diagonal blocks (jt >= 4*ic) only compute
                    # the surviving i >= jt columns; the single partial
                    # [128,128] block gets a triangular mask.  The previous
                    # chunk's projection blocks are paced into the j-loop.
                    for hp in range(HG // 2):
                        hs = (2 * hp, 2 * hp + 1)
                        ps_ys = {h: pp2.tile([P, 512], F32, tag="py", bufs=2,
                                             name=f"ps_y{h}") for h in hs}
                        ps_ls = {h: pp2.tile([P, 512], F32, tag="pl", bufs=2,
                                             name=f"ps_l{h}") for h in hs}

                        def sc_exp(jt, h):
                            r = jt - 4 * ic
                            off = max(0, r) * P
                            ps_s = pp2.tile([P, 512], F32, tag="ps", bufs=2)
                            nc.tensor.matmul(
                                ps_s[:, off:], kt_sb[:, jt, h, :],
                                qt_sb[:, 4 * ic + max(0, r):4 * (ic + 1), h, :],
                                start=True, stop=True)
                            p_sb = ph2.tile([P, 512], BF16, tag="p", bufs=6)
                            nc.scalar.activation(
                                out=p_sb[:, off:], in_=ps_s[:, off:], func=AF.Exp,
                                scale=rk_sb[:, jt, h:h + 1])
                            if r >= 0:
                                nc.gpsimd.affine_select(
                                    out=p_sb[:, off:off + P],
                                    in_=p_sb[:, off:off + P],
                                    pattern=[[1, P]], channel_multiplier=-1,
                                    base=0,
                                    compare_op=mybir.AluOpType.is_ge, fill=0.0)
                            return p_sb, off

                        ps = {h: sc_exp(0, h) for h in hs}
                        for jt in range(nj):
                            nxt = {}
                            if jt + 1 < nj:
                                nxt = {h: sc_exp(jt + 1, h) for h in hs}
                            for h in hs:
                                st, sp = jt == 0, jt == nj - 1
                                p_sb, off = ps[h]
                                nc.tensor.matmul(ps_ls[h][:, off:], ones_t,
                                                 p_sb[:, off:],
                                                 start=st, stop=sp)
                                nc.tensor.matmul(
                                    ps_ys[h][:, off:],
                                    v_sb[:, jt, h * P:(h + 1) * P],
                                    p_sb[:, off:], start=st, stop=sp)
                            ps = nxt
                            # pace the previous chunk's projection through
                            step_idx += 1
                            if step_idx > delay and pending:
                                rem_steps = 2 * nj - step_idx + 1
                                n_emit = -(-len(pending) // rem_steps)
                                for _ in range(n_emit):
                                    pending.pop(0)()
                        # Fast PSUM evacuation: 1/l = exp(-ln(l)) on ScalarE
                        # (the DVE RECIPROCAL measures 3.3us -- the ln both
                        # frees the l bank and feeds the cheap exp; both fns
                        # live in the natural_log_exp_and_others table set
                        # alongside the softmax exp, so no table thrash).
                        # DVE copies free the y banks; the normalize multiply
                        # runs behind -- nothing reads y_sb until the paced
                        # projection several steps later.  The very last pair
                        # skips the y copies (no next pair needs the banks).
                        tail = ic == NI - 1 and hp == HG // 2 - 1
                        llns, y_raws = {}, {}
                        for h in hs:
                            llns[h] = ph2.tile([P, 512], F32, tag="lln",
                                               name=f"lln{h}")
                            nc.scalar.activation(out=llns[h], in_=ps_ls[h],
                                                 func=AF.Ln)
                            if not tail:
                                y_raws[h] = ph2.tile([P, 512], BF16,
                                                     tag="y_raw",
                                                     name=f"y_raw{h}")
                                nc.vector.tensor_copy(out=y_raws[h],
                                                      in_=ps_ys[h])
                        with nc.allow_low_precision(reason="2e-2 tolerance"):
                            for h in hs:
                                linv = ph2.tile([P, 512], BF16, tag="linv")
                                nc.scalar.activation(out=linv, in_=llns[h],
                                                     func=AF.Exp, scale=-1.0)
                                nc.vector.tensor_mul(
                                    y_sb[:, h, :],
                                    ps_ys[h] if tail else y_raws[h], linv)
                    prev_y = y_sb
                for emit in pending:
                    emit()
                for emit in proj_groups(NI - 1, prev_y):
                    emit()
    _split_excess_waits(nc)
    return nc


_NC_CACHE = {}
_RUN_KWARGS = {}      # test harness hook: e.g. {"trace": True}
_LAST_RESULT = None   # BassKernelResults of the most recent run


def _rotary_tables():
    freq = (1.0 / 1024.0) ** np.linspace(0.0, 1.0, NFREQ, dtype=np.float32)
    theta = np.arange(T, dtype=np.float32)[:, None] * freq[None, :]      # [T, 32]
    cos = np.cos(theta).astype(np.float32)
    sin = np.sin(theta).astype(np.float32)
    cosb = np.tile(cos, (1, HG)).astype(ml_dtypes.bfloat16)              # [T, 128]
    sinb = np.tile(sin, (1, HG)).astype(ml_dtypes.bfloat16)
    return np.ascontiguousarray(cosb), np.ascontiguousarray(sinb)


def kernel(x, Wq, bq, Wk, bk, Wv, bv, Wp, bp):
    x = np.asarray(x, np.float32)
    Wq, Wk, Wv, Wp = (np.asarray(a, np.float32) for a in (Wq, Wk, Wv, Wp))
    bq, bk, bv, bp = (np.asarray(a, np.float32) for a in (bq, bk, bv, bp))

    has_bias = bool(np.any(bq) or np.any(bk) or np.any(bv))
    if has_bias not in _NC_CACHE:
        _NC_CACHE[has_bias] = _build_nc(has_bias)
    nc = _NC_CACHE[has_bias]

    def b16(a):
        return np.ascontiguousarray(a.astype(ml_dtypes.bfloat16))

    cosb, sinb = _rotary_tables()
    in_maps = []
    for c in range(NCORES):
        b, g = divmod(c, NCORES // B)
        sl = slice(g * GD, (g + 1) * GD)
        m = {
            "xt": b16(x[b].T),
            "wqt": b16(Wq[sl, :].T),
            "wkt": b16(Wk[sl, :].T),
            "wvt": b16(Wv[sl, :].T),
            "wpt": b16(Wp[:, sl].T),
            "cosb": cosb,
            "sinb": sinb,
        }
        if has_bias:
            m["bq"] = np.ascontiguousarray(bq[sl])
            m["bk"] = np.ascontiguousarray(bk[sl])
            m["bv"] = np.ascontiguousarray(bv[sl])
        in_maps.append(m)

    res = run_bass_kernel_spmd(nc, in_maps, list(range(NCORES)), **_RUN_KWARGS)
    global _LAST_RESULT
    _LAST_RESULT = res
    out = np.zeros((B, T, DIM), np.float32)
    for c in range(NCORES):
        out[c // (NCORES // B)] += res.results[c]["out"].astype(np.float32)
    out += bp[None, None, :]
    return out


# revision 19
# speedup vs baseline: 1.3475x; 1.0028x over previous
"""Trainium2 Bass kernel for causal self-attention with QK RMS-norm + rotary.

Full (unsharded) inputs in, full output out.  Internally sharded over 8
NeuronCores: data parallel on batch (2) x tensor parallel on head groups
(16 heads -> 4 groups of 4).  Each core computes q/k/v for its 4 heads on
its batch, causal flash-style attention, and a partial output projection
(its 512-column slice of Wp's input dim); the host sums the 4 partials per
batch ("all-reduce after proj" done host-side) and adds the output bias.

All matmul operands are bf16 (fp32 PSUM accumulation): same PE rate as
fp32r but with fast weight loads, half the DMA/SBUF traffic, and 2x DVE
throughput on SBUF-resident elementwise work.  Numerics validated against
the fp32 reference at ~3e-3 max-rel-err (gate 2e-2).

Per-core pipeline (single Bass program, SPMD over 8 cores):
  Phase 1, per 128-row t-tile: QKV projections with x^T tiles stationary
    (q/k/v share each weight load); ScalarE evacuates PSUM to bf16 SBUF;
    RMS stats are taken on the *raw* q/k (rotation preserves row norms) so
    they overlap the rotary; rotary touches only the 64 nonzero-frequency
    channels per head (the other half is pass-through, already in place
    from the copy); q scaled by 1/rms in place; k's norm is folded into
    the softmax exp scale (0.12/rms(k_j) per scores^T partition); q^T/k^T
    built with bf16 PE transposes and kept in SBUF (no DRAM spill).
  Phase 2, per 512-column query chunk, per head: scores^T = k^T-tile @ q^T
    so the softmax denominator comes from a ones-stationary matmul and
    attn@v needs no transpose of p; exp on ScalarE (bf16 out); causal
    diagonal blocks run at reduced moving size (only j<=i columns) with a
    GpSimd triangular affine_select on the single partial [128,128] block;
    y^T and l accumulate in PSUM over j-tiles; 1/l via the fast DVE
    reciprocal; output projection contracts the 4 head-slices of y^T
    against Wp^T, interleaved with the next chunk's attention; partial
    outputs written bf16 and reduced host-side.
"""

import os
import sys

import numpy as np

try:
    import concourse.bass as bass
except ImportError:  # fall back to the repo checkout baked into the image
    for _p in ("/opt/trn_rl_repo", "/root/.axon_site/_ro/trn_rl_repo"):
        if os.path.isdir(_p) and _p not in sys.path:
            sys.path.append(_p)
    import concourse.bass as bass

import ml_dtypes

import concourse.mybir as mybir
import concourse.tile as tile
from concourse.bass_utils import run_bass_kernel_spmd
from concourse.masks import make_identity
from concourse.vector_clock import ScopedClock

F32 = mybir.dt.float32
BF16 = mybir.dt.bfloat16
AF = mybir.ActivationFunctionType

DIM = 2048
HEAD_DIM = 128
NUM_HEADS = 16
B, T = 2, 2048
EPS = 1.1920929e-07
SCALE = 0.12

NCORES = 8
HG = 4                    # heads per core
GD = HG * HEAD_DIM        # 512: per-core q/k/v width and Wp input slice
NT = T // 128             # 16 t-tiles
ND = DIM // 128           # 16 contraction tiles
NI = T // 512             # 4 query chunks
P = 128
NFREQ = HEAD_DIM // 4     # 32 nonzero-frequency channels per head


class _TC(tile.TileContext):
    """TileContext whose final drain splits its semaphore waits across
    single-wait NOPs -- the walrus build in this image rejects CTRL
    instructions carrying 3+ sync waits ("Too many sync wait commands")."""

    def _drain_and_barrier(self, tick_clock, wait_clock):
        probe = self.nc.sync.nop(nofuse=True)
        wait_clock.add_sem_waits(probe.ins, ScopedClock({None: tick_clock.global_clock}))
        si = probe.ins.sync_info
        waits = list(si.on_wait) if si and si.on_wait else []
        if si is not None and si.on_wait:
            del si.on_wait[1:]
        for w in waits[1:]:
            nop = self.nc.sync.nop(nofuse=True)
            nsi = nop.ins.sync_info
            if nsi is None:
                nop.ins.sync_info = mybir.SyncInfo(on_wait=[w], on_update=[])
            else:
                nsi.on_wait.append(w)
        self.nc.sync.drain()
        self.nc.all_engine_barrier()
        assert self.sems is not None
        popped = self.nc._tile_sem_poison_stack.pop()
        assert popped is self._sem_poison
        self.nc.clear_and_free_semaphores(list(self.sems.allocated().values()))
        self.nc.all_engine_barrier()


_MAX_WAITS = 1


def _split_excess_waits(nc, maxw=_MAX_WAITS):
    """The walrus build in this image rejects instructions with >1 sync
    waits; spill extra waits onto NoOps inserted just before the offender
    on the same engine (all waits are preconditions, so order is free)."""
    n = 0
    for f in nc.m.functions:
        for bb in f.blocks:
            out = []
            for inst in bb.instructions:
                si = inst.sync_info
                waits = list(si.on_wait) if si and si.on_wait else []
                if len(waits) > maxw:
                    extra = waits[:-maxw]
                    del si.on_wait[: len(extra)]
                    for i in range(0, len(extra), maxw):
                        n += 1
                        nop = mybir.InstNoOp(name=f"I-wsplit-{n}-{inst.name}",
                                             ins=[], outs=[])
                        nop.engine = inst.engine
                        nop.sync_info = mybir.SyncInfo(
                            on_wait=extra[i:i + maxw], on_update=[])
                        out.append(nop)
                out.append(inst)
            bb.instructions[:] = out


def _build_nc(has_qkv_bias: bool):
    nc = bass.Bass("TRN2", target_bir_lowering=False, debug=False, num_devices=NCORES)

    xt = nc.dram_tensor("xt", [DIM, T], BF16, kind="ExternalInput")
    wqt = nc.dram_tensor("wqt", [DIM, GD], BF16, kind="ExternalInput")
    wkt = nc.dram_tensor("wkt", [DIM, GD], BF16, kind="ExternalInput")
    wvt = nc.dram_tensor("wvt", [DIM, GD], BF16, kind="ExternalInput")
    wpt = nc.dram_tensor("wpt", [GD, DIM], BF16, kind="ExternalInput")
    cosb = nc.dram_tensor("cosb", [T, HG * NFREQ], BF16, kind="ExternalInput")
    sinb = nc.dram_tensor("sinb", [T, HG * NFREQ], BF16, kind="ExternalInput")
    if has_qkv_bias:
        bq = nc.dram_tensor("bq", [GD], F32, kind="ExternalInput")
        bk = nc.dram_tensor("bk", [GD], F32, kind="ExternalInput")
        bv = nc.dram_tensor("bv", [GD], F32, kind="ExternalInput")
    out = nc.dram_tensor("out", [T, DIM], BF16, kind="ExternalOutput")

    xt_v = xt.rearrange("(do p) t -> p do t", p=P)      # [128, 16, 2048]
    wqt_v = wqt.rearrange("(do p) o -> p do o", p=P)    # [128, 16, 512]
    wkt_v = wkt.rearrange("(do p) o -> p do o", p=P)
    wvt_v = wvt.rearrange("(do p) o -> p do o", p=P)
    wpt_v = wpt.rearrange("(co p) o -> p co o", p=P)    # [128, 4, 2048]

    with _TC(nc) as tc:
        with (
            tc.tile_pool(name="const", bufs=1) as constp,
            tc.tile_pool(name="persist", bufs=1) as persist,
        ):
            identity = constp.tile([P, P], BF16)
            make_identity(nc, identity)
            ones_t = constp.tile([P, P], BF16)
            nc.vector.memset(ones_t, 1.0)
            eps_t = constp.tile([P, 1], F32)
            nc.vector.memset(eps_t, EPS)
            if has_qkv_bias:
                bias_b = constp.tile([P, 3, GD], F32)
                for bi, bten in enumerate((bq, bk, bv)):
                    bcast = bass.AP(tensor=bten.tensor, offset=bten.offset,
                                    ap=[[0, P]] + list(bten.ap))
                    nc.sync.dma_start(out=bias_b[:, bi, :], in_=bcast)

            v_sb = persist.tile([P, NT, GD], BF16)       # v, natural [t, head*128]
            rk_sb = persist.tile([P, NT, HG], F32)       # 0.12/rms(k) per (t, head)
            qt_sb = persist.tile([P, NT, HG, P], BF16)   # q^T [c, tt, h, t]
            kt_sb = persist.tile([P, NT, HG, P], BF16)
            wp_sb = persist.tile([P, HG, DIM], BF16)

            # ---------------- Phase 1: QKV + rotary + norms + transposes ------
            # K/V for t-tile i and Q for t-tile i-1 per iteration: Q trails
            # one tile so the PE starts on K/V as soon as the first weight
            # chunks land instead of waiting for all three weight matrices.
            with (
                tc.tile_pool(name="wqkv", bufs=1) as wpool,
                tc.tile_pool(name="ph1", bufs=3) as ph1,
                tc.tile_pool(name="ph1t", bufs=2) as ph1t,
                tc.tile_pool(name="pp1", bufs=1, space="PSUM") as pp1,
            ):
                wq_sb = wpool.tile([P, ND, GD], BF16)
                wk_sb = wpool.tile([P, ND, GD], BF16)
                wv_sb = wpool.tile([P, ND, GD], BF16)

                acts = {}

                def load_acts(tt):
                    xtile = ph1.tile([P, ND, P], BF16, tag="xtile",
                                     name=f"xtile{tt}")
                    if tt == 0:
                        # split so the first matmul gates on 128KB, not 512KB
                        nc.gpsimd.dma_start(out=xtile[:, 0:4, :],
                                            in_=xt_v[:, 0:4, 0:P])
                        nc.gpsimd.dma_start(out=xtile[:, 4:, :],
                                            in_=xt_v[:, 4:, 0:P])
                    else:
                        nc.gpsimd.dma_start(out=xtile,
                                            in_=xt_v[:, :, tt * P:(tt + 1) * P])
                    ctile = ph1.tile([P, HG * NFREQ], BF16, tag="ctile",
                                     name=f"ctile{tt}")
                    stile = ph1.tile([P, HG * NFREQ], BF16, tag="stile",
                                     name=f"stile{tt}")
                    nc.gpsimd.dma_start(out=ctile, in_=cosb[tt * P:(tt + 1) * P, :])
                    nc.gpsimd.dma_start(out=stile, in_=sinb[tt * P:(tt + 1) * P, :])
                    acts[tt] = (xtile, ctile, stile)

                # Three DMA queues in parallel (wk+wv on sync, wq on scalar,
                # activations on gpsimd), few large transfers (the queue
                # engine costs ~600ns per dma_start) with small first chunks
                # so tile 0's contraction starts as early as possible.
                load_acts(0)
                for lo, hi in ((0, 2), (2, 6), (6, ND)):
                    nc.sync.dma_start(out=wk_sb[:, lo:hi, :],
                                      in_=wkt_v[:, lo:hi, :])
                    nc.sync.dma_start(out=wv_sb[:, lo:hi, :],
                                      in_=wvt_v[:, lo:hi, :])
                    nc.scalar.dma_start(out=wq_sb[:, lo:hi, :],
                                        in_=wqt_v[:, lo:hi, :])
                # Wp not needed until the first projection, far into phase 2.
                nc.sync.dma_start(out=wp_sb, in_=wpt_v)

                def qk_post(which, ps, tt):
                    """PSUM -> bf16 SBUF, raw-value RMS stats, in-place rotary
                    on the nonzero-frequency half of each head's channels.
                    Returns the finished [128, 512] bf16 tile."""
                    _, ctile, stile = acts[tt]
                    base = ph1t.tile([P, GD], BF16, tag=which + "base",
                                     bufs=4 if which == "q" else 2,
                                     name=f"{which}base{tt}")
                    if has_qkv_bias:
                        nc.vector.tensor_add(base, ps, bias_b[:, 1 if which == "k" else 0, :])
                    else:
                        nc.scalar.copy(out=base, in_=ps)
                    # stats on the raw values: rotation preserves row norms
                    sq = ph1t.tile([P, GD], BF16, tag="sq")
                    nc.vector.tensor_mul(sq, base, base)
                    ms = ph1t.tile([P, HG], F32, tag=which + "ms")
                    nc.vector.reduce_sum(out=ms,
                                         in_=sq.rearrange("p (h c) -> p h c", h=HG),
                                         axis=mybir.AxisListType.X)
                    rstd = ph1t.tile([P, HG], F32, tag=which + "rstd")
                    nc.scalar.activation(out=rstd, in_=ms, func=AF.Sqrt,
                                         scale=1.0 / HEAD_DIM, bias=eps_t[:, 0:1])
                    if which == "q":
                        nc.vector.reciprocal(out=rstd, in_=rstd)
                    else:
                        nc.vector.reciprocal(out=rk_sb[:, tt, :], in_=rstd)
                        nc.vector.tensor_scalar_mul(
                            rk_sb[:, tt, :], rk_sb[:, tt, :], SCALE)
                    # rotary on channels [0:32] & [64:96] of each head; the
                    # zero-frequency half is pass-through (already in base)
                    br = base.rearrange("p (h c) -> p h c", h=HG)
                    x1f = br[:, :, 0:NFREQ]
                    x2f = br[:, :, 2 * NFREQ:3 * NFREQ]
                    u1 = ph1t.tile([P, HG, NFREQ], BF16, tag="u1")
                    w2 = ph1t.tile([P, HG, NFREQ], BF16, tag="w2")
                    u2 = ph1t.tile([P, HG, NFREQ], BF16, tag="u2")
                    w1 = ph1t.tile([P, HG, NFREQ], BF16, tag="w1")
                    cr = ctile.rearrange("p (h c) -> p h c", h=HG)
                    sr = stile.rearrange("p (h c) -> p h c", h=HG)
                    nc.vector.tensor_mul(u1, x1f, cr)
                    nc.vector.tensor_mul(w2, x2f, sr)
                    nc.vector.tensor_mul(u2, x2f, cr)
                    nc.vector.tensor_mul(w1, x1f, sr)
                    nc.vector.tensor_add(x1f, u1, w2)
                    nc.vector.tensor_sub(x2f, u2, w1)
                    if which == "q":
                        for h in range(HG):
                            nc.vector.tensor_scalar_mul(
                                br[:, h, :], br[:, h, :], rstd[:, h:h + 1])
                    return base

                def qk_transpose(which, base, tt):
                    dst = qt_sb if which == "q" else kt_sb
                    for h in range(HG):
                        ptr = pp1.tile([P, P], BF16, tag="ptr", bufs=2)
                        nc.tensor.transpose(ptr, base[:, h * P:(h + 1) * P],
                                            identity)
                        if h % 2 == 0:
                            nc.scalar.copy(out=dst[:, tt, h, :], in_=ptr)
                        else:
                            nc.vector.tensor_copy(out=dst[:, tt, h, :], in_=ptr)

                # Software pipeline: per iteration i the PE runs K/V(i) then
                # Q(i-1), then transposes k(i) (whose post chain overlapped
                # the Q matmuls) and q(i-2) (chain finished an iteration ago).
                qbases = {}
                for i in range(NT + 2):
                    if 2 <= i:
                        qk_transpose("q", qbases.pop(i - 2), i - 2)
                    if i < NT:
                        if i + 1 < NT:
                            load_acts(i + 1)
                        xtile = acts[i][0]
                        ps_k = pp1.tile([P, GD], F32, tag="psk", bufs=2)
                        ps_v = pp1.tile([P, GD], F32, tag="psv", bufs=2)
                        for d in range(ND):
                            st, sp = d == 0, d == ND - 1
                            nc.tensor.matmul(ps_k, xtile[:, d, :], wk_sb[:, d, :],
                                             start=st, stop=sp)
                            nc.tensor.matmul(ps_v, xtile[:, d, :], wv_sb[:, d, :],
                                             start=st, stop=sp)
                        if has_qkv_bias:
                            nc.vector.tensor_add(v_sb[:, i, :], ps_v, bias_b[:, 2, :])
                        else:
                            nc.scalar.copy(out=v_sb[:, i, :], in_=ps_v)
                        kbase = qk_post("k", ps_k, i)
                    if 1 <= i <= NT:
                        tt = i - 1
                        xtile = acts[tt][0]
                        ps_q = pp1.tile([P, GD], F32, tag="psq", bufs=2)
                        for d in range(ND):
                            nc.tensor.matmul(ps_q, xtile[:, d, :], wq_sb[:, d, :],
                                             start=d == 0, stop=d == ND - 1)
                        qbases[tt] = qk_post("q", ps_q, tt)
                    if i < NT:
                        qk_transpose("k", kbase, i)
                # Preload the exp/ln activation table while the PE drains the
                # final transposes -- otherwise phase 2's first exp eats the
                # ~2.7us ACT_TABLE_LOAD on the critical path.
                dummy = ph1t.tile([P, 1], F32, tag="dummy")
                nc.scalar.activation(out=dummy, in_=eps_t, func=AF.Ln)

            # ---------------- Phase 2: attention + projection -----------------
            with (
                tc.tile_pool(name="ph2", bufs=2) as ph2,
                tc.tile_pool(name="pp2", bufs=1, space="PSUM") as pp2,
            ):
                def proj_groups(ic, y_sb):
                    """16 closures, each emitting one 512-col output block:
                    4 accumulating matmuls + a DVE evacuation + the DMA out.
                    Paced into the next chunk's attention loop so po banks
                    never stall the PE and the output DMA spreads out."""
                    def mk(it, dc):
                        def emit():
                            ps_o = pp2.tile([P, 512], F32, tag="po", bufs=2)
                            for co in range(HG):
                                nc.tensor.matmul(
                                    ps_o, y_sb[:, co, it * P:(it + 1) * P],
                                    wp_sb[:, co, dc * 512:(dc + 1) * 512],
                                    start=co == 0, stop=co == HG - 1)
                            o_sb = ph2.tile([P, 512], BF16, tag="o", bufs=3)
                            nc.vector.tensor_copy(out=o_sb, in_=ps_o)
                            nc.sync.dma_start(
                                out=out[(4 * ic + it) * P:(4 * ic + it + 1) * P,
                                        dc * 512:(dc + 1) * 512],
                                in_=o_sb)
                        return emit
                    return [mk(it, dc) for it in range(4) for dc in range(4)]

                pending = []
                prev_y = None
                for ic in range(NI):
                    if prev_y is not None:
                        pending = proj_groups(ic - 1, prev_y)
                    y_sb = ph2.tile([P, HG, 512], BF16, tag="y")
                    nj = 4 * (ic + 1)
                    # Projection blocks read all 4 heads of the previous
                    # chunk's y; its last pair finishes ~3us into this chunk,
                    # so hold pacing for the first 3 steps (head-of-line
                    # blocking on the in-order PE queue otherwise).
                    step_idx = 0
                    delay = 7
                    # Heads in interleaved pairs with scores+exp prefetched one
                    # j-step ahead.  Diagonal blocks (jt >= 4*ic) only compute
                    # the surviving i >= jt columns; the single partial
                    # [128,128] block gets a triangular mask.  The previous
                    # chunk's projection blocks are paced into the j-loop.
                    for hp in range(HG // 2):
                        hs = (2 * hp, 2 * hp + 1)
                        ps_ys = {h: pp2.tile([P, 512], F32, tag="py", bufs=2,
                                             name=f"ps_y{h}") for h in hs}
                        ps_ls = {h: pp2.tile([P, 512], F32, tag="pl", bufs=2,
                                             name=f"ps_l{h}") for h in hs}

                        def sc_exp(jt, h):
                            r = jt - 4 * ic
                            off = max(0, r) * P
                            ps_s = pp2.tile([P, 512], F32, tag="ps", bufs=2)
                            nc.tensor.matmul(
                                ps_s[:, off:], kt_sb[:, jt, h, :],
                                qt_sb[:, 4 * ic + max(0, r):4 * (ic + 1), h, :],
                                start=True, stop=True)
                            p_sb = ph2.tile([P, 512], BF16, tag="p", bufs=6)
                            nc.scalar.activation(
                                out=p_sb[:, off:], in_=ps_s[:, off:], func=AF.Exp,
                                scale=rk_sb[:, jt, h:h + 1])
                            if r >= 0:
                                nc.gpsimd.affine_select(
                                    out=p_sb[:, off:off + P],
                                    in_=p_sb[:, off:off + P],
                                    pattern=[[1, P]], channel_multiplier=-1,
                                    base=0,
                                    compare_op=mybir.AluOpType.is_ge, fill=0.0)
                            return p_sb, off

                        ps = {h: sc_exp(0, h) for h in hs}
                        for jt in range(nj):
                            nxt = {}
                            if jt + 1 < nj:
                                nxt = {h: sc_exp(jt + 1, h) for h in hs}
                            for h in hs:
                                st, sp = jt == 0, jt == nj - 1
                                p_sb, off = ps[h]
                                nc.tensor.matmul(ps_ls[h][:, off:], ones_t,
                                                 p_sb[:, off:],
                                                 start=st, stop=sp)
                                nc.tensor.matmul(
                                    ps_ys[h][:, off:],
                                    v_sb[:, jt, h * P:(h + 1) * P],
                                    p_sb[:, off:], start=st, stop=sp)
                            ps = nxt
                            # pace the previous chunk's projection through
                            step_idx += 1
                            if step_idx > delay and pending:
                                rem_steps = 2 * nj - step_idx + 1
                                n_emit = -(-len(pending) // rem_steps)
                                for _ in range(n_emit):
                                    pending.pop(0)()
                        # Fast PSUM evacuation: 1/l = exp(-ln(l)) on ScalarE
                        # (the DVE RECIPROCAL measures 3.3us -- the ln both
                        # frees the l bank and feeds the cheap exp; both fns
                        # live in the natural_log_exp_and_others table set
                        # alongside the softmax exp, so no table thrash).
                        # DVE copies free the y banks; the normalize multiply
                        # runs behind -- nothing reads y_sb until the paced
                        # projection several steps later.  The very last pair
                        # skips the y copies (no next pair needs the banks).
                        tail = ic == NI - 1 and hp == HG // 2 - 1
                        llns, y_raws = {}, {}
                        for h in hs:
                            llns[h] = ph2.tile([P, 512], F32, tag="lln",
                                               name=f"lln{h}")
                            nc.scalar.activation(out=llns[h], in_=ps_ls[h],
                                                 func=AF.Ln)
                            if not tail:
                                y_raws[h] = ph2.tile([P, 512], BF16,
                                                     tag="y_raw",
                                                     name=f"y_raw{h}")
                                nc.vector.tensor_copy(out=y_raws[h],
                                                      in_=ps_ys[h])
                        with nc.allow_low_precision(reason="2e-2 tolerance"):
                            for h in hs:
                                linv = ph2.tile([P, 512], BF16, tag="linv")
                                nc.scalar.activation(out=linv, in_=llns[h],
                                                     func=AF.Exp, scale=-1.0)
                                nc.vector.tensor_mul(
                                    y_sb[:, h, :],
                                    ps_ys[h] if tail else y_raws[h], linv)
                    prev_y = y_sb
                for emit in pending:
                    emit()
                for emit in proj_groups(NI - 1, prev_y):
                    emit()
    _split_excess_waits(nc)
    return nc


_NC_CACHE = {}
_RUN_KWARGS = {}      # test harness hook: e.g. {"trace": True}
_LAST_RESULT = None   # BassKernelResults of the most recent run


def _rotary_tables():
    freq = (1.0 / 1024.0) ** np.linspace(0.0, 1.0, NFREQ, dtype=np.float32)
    theta = np.arange(T, dtype=np.float32)[:, None] * freq[None, :]      # [T, 32]
    cos = np.cos(theta).astype(np.float32)
    sin = np.sin(theta).astype(np.float32)
    cosb = np.tile(cos, (1, HG)).astype(ml_dtypes.bfloat16)              # [T, 128]
    sinb = np.tile(sin, (1, HG)).astype(ml_dtypes.bfloat16)
    return np.ascontiguousarray(cosb), np.ascontiguousarray(sinb)


def kernel(x, Wq, bq, Wk, bk, Wv, bv, Wp, bp):
    x = np.asarray(x, np.float32)
    Wq, Wk, Wv, Wp = (np.asarray(a, np.float32) for a in (Wq, Wk, Wv, Wp))
    bq, bk, bv, bp = (np.asarray(a, np.float32) for a in (bq, bk, bv, bp))

    has_bias = bool(np.any(bq) or np.any(bk) or np.any(bv))
    if has_bias not in _NC_CACHE:
        _NC_CACHE[has_bias] = _build_nc(has_bias)
    nc = _NC_CACHE[has_bias]

    def b16(a):
        return np.ascontiguousarray(a.astype(ml_dtypes.bfloat16))

    cosb, sinb = _rotary_tables()
    in_maps = []
    for c in range(NCORES):
        b, g = divmod(c, NCORES // B)
        sl = slice(g * GD, (g + 1) * GD)
        m = {
            "xt": b16(x[b].T),
            "wqt": b16(Wq[sl, :].T),
            "wkt": b16(Wk[sl, :].T),
            "wvt": b16(Wv[sl, :].T),
            "wpt": b16(Wp[:, sl].T),
            "cosb": cosb,
            "sinb": sinb,
        }
        if has_bias:
            m["bq"] = np.ascontiguousarray(bq[sl])
            m["bk"] = np.ascontiguousarray(bk[sl])
            m["bv"] = np.ascontiguousarray(bv[sl])
        in_maps.append(m)

    res = run_bass_kernel_spmd(nc, in_maps, list(range(NCORES)), **_RUN_KWARGS)
    global _LAST_RESULT
    _LAST_RESULT = res
    out = np.zeros((B, T, DIM), np.float32)
    for c in range(NCORES):
        out[c // (NCORES // B)] += res.results[c]["out"].astype(np.float32)
    out += bp[None, None, :]
    return out
